# revision 2
# baseline (speedup 1.0000x reference)
"""Trainium2 Bass kernel for nn_NeuralNetwork_S (kwta / topk_masking).

Strategy:
- Pure data parallel over 8 NeuronCores: 2048 rows each, 4 groups of 512 rows.
- All matmuls fp32-grade via 3-term float32r split (12-bit hi + lo parts):
  x@w = x_hi@w_hi + x_hi@w_lo + x_lo@w_hi   (err ~1.8e-7, 3 cyc/row vs fp32's 4)
- cx chains: softmax is monotone -> k = argmax(logits) via vector.max/max_index.
- kwta: per-row (k+1)-th largest value u found by batched fp32 bisection on
  count(z > mid) computed exactly on ACT (sigmoid with power-of-two scale 2^100
  saturates to an exact 0/1 step), then Max8 of the final interval + select by
  rank; mask = z > u; losers multiplied by (1/3).
"""

import sys
import os

_TRN = "/opt/trn_rl_repo"
if _TRN not in sys.path:
    sys.path.insert(0, _TRN)

import numpy as np
import concourse.bass as bass
import concourse.mybir as mybir
import concourse.tile as tile
from concourse import bacc
from concourse.bass_utils import run_bass_kernel_spmd
from concourse.masks import make_identity

P = 128
B = 16384
NCORES = 8
BC = B // NCORES          # 2048 rows per core
BG = 512                  # rows per group
NG = BC // BG             # 4 groups
GT = BG // P              # 4 tiles of 128 rows per group
IN = 1028
INP = 1152                # padded to 9*128
HID = 1024
HID2 = 512
HEADS = 128

F32 = mybir.dt.float32
F32R = mybir.dt.float32r
BF16 = mybir.dt.bfloat16
U32 = mybir.dt.uint32
AF = mybir.ActivationFunctionType
OP = mybir.AluOpType
AX = mybir.AxisListType

SCALE = float(2.0 ** 100)   # power of two -> ACT affine is exact, step is exact
ITERS = {1024: 12, 512: 12, 128: 10}
THIRD = 1.0 / 3.0
TWO3 = 2.0 / 3.0


def rne12(x):
    m, e = np.frexp(x)
    s = 2.0 ** 12
    return np.ldexp(np.round(m * s) / s, e).astype(np.float32)


def _pad_k(a, kdim):
    """Pad leading dim of [K, N] up to multiple of 128."""
    k = a.shape[0]
    kp = ((k + P - 1) // P) * P
    if kp == k:
        return np.ascontiguousarray(a)
    out = np.zeros((kp, a.shape[1]), dtype=a.dtype)
    out[:k] = a
    return out


# ----------------------------------------------------------------------------
# program builder
# ----------------------------------------------------------------------------

def build_program():
    nc = bacc.Bacc("TRN2", target_bir_lowering=False, debug=False)

    d = {}

    def din(name, shape, dt=F32R):
        d[name] = nc.dram_tensor(name, list(shape), dt, kind="ExternalInput")
        return d[name]

    # per-core activations (column-sliced by host)
    din("ciT_hi", [INP, BC])
    din("ciT_lo", [INP, BC])
    # weights (replicated): wT padded [Kpad, out], hi/lo
    wk = {}
    for name, i, o in [
        ("cx11", IN, HID), ("cx12", HID, HID), ("cx21", IN, HID2),
        ("cx22", HID2, HID2), ("cx31", IN, HEADS), ("cx32", HEADS, HEADS),
        ("l1", IN, HID), ("l2", HID, HID2), ("l3", HID2, HEADS),
        ("l4", HEADS, HEADS),
    ]:
        kp = ((i + P - 1) // P) * P
        wk[name] = (kp // P, o)
        din(f"{name}_hi", [kp, o])
        din(f"{name}_lo", [kp, o])
    # biases: replicated [P, out] for (b)-layers; column [P, chunks] for (a)
    for name, o in [("cx12", HID), ("cx22", HID2), ("cx32", HEADS),
                    ("l1", HID), ("l2", HID2)]:
        din(f"{name}_brep", [P, o], F32)
    for name, mch in [("cx11", HID // P), ("cx21", HID2 // P),
                      ("cx31", 1), ("l3", 1), ("l4", 1)]:
        din(f"{name}_bcol", [P, mch], F32)

    outT = nc.dram_tensor("outT", [P, BC], F32, kind="ExternalOutput")
    dbg = {}
    if os.environ.get("KERNEL_DEBUG"):
        dbg["z1"] = nc.dram_tensor("dbg_z1", [P, GT * HID], F32, kind="ExternalOutput")
        dbg["h1hi"] = nc.dram_tensor("dbg_h1hi", [P, (HID // P) * BG], F32R, kind="ExternalOutput")
        dbg["zcx0"] = nc.dram_tensor("dbg_zcx0", [P, GT * HID], F32, kind="ExternalOutput")
        dbg["kk"] = nc.dram_tensor("dbg_kk", [P, 3 * GT], F32, kind="ExternalOutput")
        dbg["ulo"] = nc.dram_tensor("dbg_ulo", [P, 4 * GT], F32, kind="ExternalOutput")
        dbg["x1"] = nc.dram_tensor("dbg_x1", [P, GT * HID], F32, kind="ExternalOutput")

    with tile.TileContext(nc) as tc:
        _emit(tc, nc, d, wk, outT, dbg)
    nc.compile()
    return nc


def _emit(tc, nc, d, wk, outT, dbg):
    import contextlib

    ctx = contextlib.ExitStack()
    with ctx:
        act = ctx.enter_context(tc.tile_pool(name="act", bufs=1))
        ci_pool = ctx.enter_context(tc.tile_pool(name="ci", bufs=1))
        wpool = ctx.enter_context(tc.tile_pool(name="w", bufs=3))
        small = ctx.enter_context(tc.tile_pool(name="small", bufs=1))
        scratch = ctx.enter_context(tc.tile_pool(name="scratch", bufs=1))
        psb = ctx.enter_context(tc.tile_pool(name="psb", bufs=4, space="PSUM"))
        psa = ctx.enter_context(tc.tile_pool(name="psa", bufs=2, space="PSUM"))
        pst = ctx.enter_context(tc.tile_pool(name="pst", bufs=2, space="PSUM"))

        ident = act.tile([P, P], F32, tag="ident")
        make_identity(nc, ident[:])
        negbig = act.tile([P, 1], F32, tag="negbig")
        nc.vector.memset(negbig[:], -1.0e30)
        iota8 = act.tile([P, 8], F32, tag="iota8")
        iota8u = act.tile([P, 8], U32, tag="iota8u")
        nc.gpsimd.iota(iota8u[:], pattern=[[1, 8]], base=0, channel_multiplier=0)
        nc.vector.tensor_copy(iota8[:], iota8u[:])

        # stream a weight tile [P, kchunks, width] slab
        def wtile(name, part, kcs, c0, o0, width, tag):
            t = wpool.tile([P, len(kcs), width], F32R, tag=tag)
            src = d[f"{name}_{part}"].rearrange("(c p) o -> p c o", p=P)
            nc.sync.dma_start(
                t[:], src[:, c0:c0 + len(kcs), o0:o0 + width]
            )
            return t

        def mm3(ps, sh, sl, mh, ml, first, last):
            nc.tensor.matmul(ps, sh, mh, start=first, stop=False)
            nc.tensor.matmul(ps, sh, ml, start=False, stop=False)
            nc.tensor.matmul(ps, sl, mh, start=False, stop=last)

        # ---------------- kwta bisection over one group-layer ---------------
        def kwta(zg, xg, kk, n, gi, li):
            """zg: [P, GT, n] fp32; xg out same; kk [P, GT] fp32 counts.
            No instruction writes a tile it also reads (ping-pong state)."""
            I = ITERS[n]
            tg = "kw"
            # ping-pong state pairs
            loA = small.tile([P, GT], F32, tag=f"{tg}loA")
            loB = small.tile([P, GT], F32, tag=f"{tg}loB")
            hiA = small.tile([P, GT], F32, tag=f"{tg}hiA")
            hiB = small.tile([P, GT], F32, tag=f"{tg}hiB")
            chA = small.tile([P, GT], F32, tag=f"{tg}chA")
            chB = small.tile([P, GT], F32, tag=f"{tg}chB")
            cnt = small.tile([P, GT], F32, tag=f"{tg}cnt")
            kp1 = small.tile([P, GT], F32, tag=f"{tg}kp1")
            msum = small.tile([P, GT], F32, tag=f"{tg}msum")
            mid = small.tile([P, GT], F32, tag=f"{tg}mid")
            nbias = small.tile([P, GT], F32, tag=f"{tg}nb")
            mn = small.tile([P, GT], F32, tag=f"{tg}mn")
            selu = small.tile([P, GT], mybir.dt.uint8, tag=f"{tg}selu")
            trash = small.tile([P, n], BF16, tag=f"{tg}trash")

            nc.vector.tensor_scalar(kp1[:], kk[:], 1.0, None, op0=OP.add)
            nc.vector.memset(chA[:], 0.0)
            for t in range(GT):
                nc.vector.reduce_max(hiA[:, t:t + 1], zg[:, t, :], axis=AX.X)
                nc.vector.tensor_reduce(
                    out=mn[:, t:t + 1], in_=zg[:, t, :], op=OP.min, axis=AX.X
                )
            nc.vector.tensor_scalar(loA[:], mn[:], 1.0, None, op0=OP.subtract)

            lo, hi, ch = loA, hiA, chA
            lon, hin, chn = loB, hiB, chB
            for it in range(I):
                nc.vector.tensor_tensor(msum[:], lo[:], hi[:], op=OP.add)
                nc.vector.tensor_scalar(mid[:], msum[:], 0.5, None, op0=OP.mult)
                nc.vector.tensor_scalar(nbias[:], mid[:], -SCALE, None,
                                        op0=OP.mult)
                for t in range(GT):
                    nc.scalar.activation(
                        trash[:], zg[:, t, :], AF.Sigmoid,
                        bias=nbias[:, t:t + 1], scale=SCALE,
                        accum_out=cnt[:, t:t + 1],
                    )
                # sel = cnt >= k+1 -> lo=mid ; else hi=mid, chi=cnt
                nc.vector.tensor_tensor(selu[:], cnt[:], kp1[:], op=OP.is_ge)
                nc.vector.select(lon[:], selu[:], mid[:], lo[:])
                nc.vector.select(hin[:], selu[:], hi[:], mid[:])
                nc.vector.select(chn[:], selu[:], ch[:], cnt[:])
                lo, lon = lon, lo
                hi, hin = hin, hi
                ch, chn = chn, ch

            # floor(chi): kill +0.5 from exact z==mid ties (casts round-nearest)
            chii = small.tile([P, GT], mybir.dt.int32, tag=f"{tg}chii")
            nc.vector.tensor_scalar(chn[:], ch[:], 0.25, None, op0=OP.subtract)
            nc.vector.tensor_copy(chii[:], chn[:])
            nc.vector.tensor_copy(ch[:], chii[:])
            # 0-indexed rank of u within interval: rm1 = kk - chi
            rm1 = small.tile([P, GT], F32, tag=f"{tg}rm1")
            nc.vector.tensor_tensor(rm1[:], kk[:], ch[:], op=OP.subtract)

            for t in range(GT):
                m1 = scratch.tile([P, n], F32, tag=f"{tg}m1")
                gu8 = scratch.tile([P, n], mybir.dt.uint8, tag=f"{tg}gu8")
                msk = scratch.tile([P, n], F32, tag=f"{tg}msk")
                nc.vector.tensor_scalar(m1[:], zg[:, t, :], lo[:, t:t + 1],
                                        None, op0=OP.max)
                nc.vector.tensor_scalar(gu8[:], zg[:, t, :], hi[:, t:t + 1],
                                        None, op0=OP.is_gt)
                nc.vector.select(msk[:], gu8[:], negbig[:].to_broadcast([P, n]),
                                 m1[:])
                m8 = small.tile([P, 8], F32, tag=f"{tg}m8")
                nc.vector.max(out=m8[:], in_=msk[:])
                eq = small.tile([P, 8], F32, tag=f"{tg}eq")
                nc.vector.tensor_scalar(eq[:], iota8[:], rm1[:, t:t + 1],
                                        None, op0=OP.is_equal)
                pr = small.tile([P, 8], F32, tag=f"{tg}pr")
                nc.vector.tensor_tensor(pr[:], eq[:], m8[:], op=OP.mult)
                u = small.tile([P, 1], F32, tag=f"{tg}u")
                nc.vector.reduce_sum(u[:], pr[:], axis=AX.X)
                if dbg and gi == 0 and li == 1:
                    nc.sync.dma_start(dbg["ulo"][:, t:t + 1], u[:])
                    if t == 0:
                        nc.sync.dma_start(dbg["ulo"][:, GT:2 * GT], lo[:])
                        nc.sync.dma_start(dbg["ulo"][:, 2 * GT:3 * GT], hi[:])
                        nc.sync.dma_start(dbg["ulo"][:, 3 * GT:4 * GT], ch[:])
                # apply: x = (z > u) ? z : z/3
                geu = scratch.tile([P, n], mybir.dt.uint8, tag=f"{tg}gu8",
                                   name="geu")
                nc.vector.tensor_scalar(geu[:], zg[:, t, :], u[:], None,
                                        op0=OP.is_gt)
                zth = scratch.tile([P, n], F32, tag=f"{tg}m1", name="zth")
                nc.vector.tensor_scalar(zth[:], zg[:, t, :], THIRD, None,
                                        op0=OP.mult)
                nc.vector.select(xg[:, t, :], geu[:], zg[:, t, :], zth[:])

        # transpose [P, GT, n] fp32 -> xT hi/lo [P, n//P, BG] f32r
        def transpose_split(xg, xT_hi, xT_lo, n):
            nch = n // P
            for t in range(GT):
                for c0 in range(0, nch, 4):
                    cw = min(4, nch - c0)
                    ps = pst.tile([P, 4 * P], F32, tag="pstT")
                    for c in range(c0, c0 + cw):
                        nc.tensor.transpose(
                            ps[:, (c - c0) * P:(c - c0 + 1) * P],
                            xg[:, t, c * P:(c + 1) * P], ident[:],
                        )
                    xf = scratch.tile([P, 4 * P], F32, tag="sc512", name="xf")
                    nc.any.tensor_copy(xf[:, :cw * P], ps[:, :cw * P])
                    src = xf[:, :cw * P].rearrange("p (c q) -> p c q", q=P)
                    dhi = xT_hi[:, c0:c0 + cw, t * P:(t + 1) * P]
                    dlo = xT_lo[:, c0:c0 + cw, t * P:(t + 1) * P]
                    nc.vector.tensor_copy(dhi, src)
                    nc.vector.tensor_tensor(dlo, src, dhi, op=OP.subtract)

        # ---------------- per-group emission ---------------
        for g in range(NG):
            col0 = g * BG

            ciT_hi = ci_pool.tile([P, INP // P, BG], F32R, tag="ciT_hi")
            ciT_lo = ci_pool.tile([P, INP // P, BG], F32R, tag="ciT_lo")
            for part, t_ in (("hi", ciT_hi), ("lo", ciT_lo)):
                nc.sync.dma_start(
                    t_[:],
                    d[f"ciT_{part}"].rearrange("(c p) b -> p c b", p=P)[
                        :, :, col0:col0 + BG],
                )

            # ---- l1 (b): z1[t] [P, 1024] = ciT.T @ l1wT + b
            kc1 = wk["l1"][0]
            z1 = act.tile([P, GT, HID], F32, tag="zb")
            b_l1 = small.tile([P, HID], F32, tag="bias", name="b_l1")
            nc.sync.dma_start(b_l1[:], d["l1_brep"][:])
            for n0 in range(0, HID, 512):
                pss = [psb.tile([P, 512], F32, tag="psb", name=f"psb{_t}") for _t in range(GT)]
                for k in range(kc1):
                    wh = wtile("l1", "hi", [k], k, n0, 512, "wb_hi")
                    wl = wtile("l1", "lo", [k], k, n0, 512, "wb_lo")
                    for t in range(GT):
                        mm3(pss[t][:], ciT_hi[:, k, t * P:(t + 1) * P],
                            ciT_lo[:, k, t * P:(t + 1) * P],
                            wh[:, 0, :], wl[:, 0, :], k == 0, k == kc1 - 1)
                for t in range(GT):
                    nc.vector.scalar_tensor_tensor(
                        z1[:, t, n0:n0 + 512], pss[t][:], 1.0,
                        b_l1[:, n0:n0 + 512], op0=OP.mult, op1=OP.add)

            if dbg and g == 0:
                nc.sync.dma_start(dbg["z1"][:],
                                  z1[:].rearrange("p a b -> p (a b)"))
            # ---- cx chains -> kk
            kks = []
            for cn, (pre, post, hidn) in enumerate(
                [("cx11", "cx12", HID), ("cx21", "cx22", HID2),
                 ("cx31", "cx32", HEADS)]
            ):
                mch = hidn // P
                kcp = wk[pre][0]
                hT_hi = act.tile([P, mch, BG], F32R, tag=f"T_hi{cn}")
                hT_lo = act.tile([P, mch, BG], F32R, tag=f"T_lo{cn}")
                bcol = small.tile([P, mch], F32, tag=f"bcol{cn}")
                nc.sync.dma_start(bcol[:], d[f"{pre}_bcol"][:])
                for m in range(mch):
                    ps = psa.tile([P, BG], F32, tag="psa")
                    wh = wtile(pre, "hi", list(range(kcp)), 0, m * P, P,
                               f"wa_hi")
                    wl = wtile(pre, "lo", list(range(kcp)), 0, m * P, P,
                               f"wa_lo")
                    for k in range(kcp):
                        mm3(ps[:], wh[:, k, :], wl[:, k, :],
                            ciT_hi[:, k, :], ciT_lo[:, k, :],
                            k == 0, k == kcp - 1)
                    hf = scratch.tile([P, BG], F32, tag="sc512")
                    nc.scalar.activation(hf[:], ps[:], AF.Tanh,
                                         bias=bcol[:, m:m + 1], scale=1.0)
                    nc.vector.tensor_copy(hT_hi[:, m, :], hf[:])
                    nc.vector.tensor_tensor(hT_lo[:, m, :], hf[:],
                                            hT_hi[:, m, :], op=OP.subtract)
                # second layer (b): zcx [P, GT, hidn]
                zcx = act.tile([P, GT, hidn], F32, tag=f"xz{cn}")
                brep = small.tile([P, hidn], F32, tag="bias", name="brep")
                nc.sync.dma_start(brep[:], d[f"{post}_brep"][:])
                for n0 in range(0, hidn, 512):
                    nw = min(512, hidn)
                    pss = [psb.tile([P, nw], F32, tag="psb", name=f"psbx{_t}") for _t in range(GT)]
                    for k in range(mch):
                        wh = wtile(post, "hi", [k], k, n0, nw, "wb_hi")
                        wl = wtile(post, "lo", [k], k, n0, nw, "wb_lo")
                        for t in range(GT):
                            mm3(pss[t][:], hT_hi[:, k, t * P:(t + 1) * P],
                                hT_lo[:, k, t * P:(t + 1) * P],
                                wh[:, 0, :], wl[:, 0, :], k == 0, k == mch - 1)
                    for t in range(GT):
                        nc.vector.scalar_tensor_tensor(
                            zcx[:, t, n0:n0 + nw], pss[t][:], 1.0,
                            brep[:, n0:n0 + nw], op0=OP.mult, op1=OP.add)
                if dbg and g == 0 and cn == 0:
                    nc.sync.dma_start(dbg["h1hi"][:],
                                      hT_hi[:].rearrange("p a b -> p (a b)"))
                    nc.sync.dma_start(dbg["zcx0"][:],
                                      zcx[:].rearrange("p a b -> p (a b)"))
                kk = small.tile([P, GT], F32, tag=f"kk{cn}")
                m8 = small.tile([P, 8], F32, tag="am8")
                idx = small.tile([P, 8], U32, tag="aidx")
                for t in range(GT):
                    nc.vector.max(out=m8[:], in_=zcx[:, t, :])
                    nc.vector.max_index(idx[:], m8[:], zcx[:, t, :])
                    nc.vector.tensor_copy(kk[:, t:t + 1], idx[:, 0:1])
                if dbg and g == 0:
                    nc.sync.dma_start(dbg["kk"][:, cn * GT:(cn + 1) * GT], kk[:])
                kks.append(kk)

            # ---- kwta1 -> x1, transpose/split
            x1 = act.tile([P, GT, HID], F32, tag="xz0", name="x1")
            kwta(z1, x1, kks[0], HID, g, 1)
            if dbg and g == 0:
                nc.sync.dma_start(dbg["x1"][:],
                                  x1[:].rearrange("p a b -> p (a b)"))
            x1T_hi = act.tile([P, HID // P, BG], F32R, tag="T_hi0", name="x1T_hi")
            x1T_lo = act.tile([P, HID // P, BG], F32R, tag="T_lo0", name="x1T_lo")
            transpose_split(x1, x1T_hi, x1T_lo, HID)

            # ---- l2 (b): z2 [P, GT, 512]
            z2 = act.tile([P, GT, HID2], F32, tag="zb", name="z2")
            b_l2 = small.tile([P, HID2], F32, tag="bias", name="b_l2")
            nc.sync.dma_start(b_l2[:], d["l2_brep"][:])
            pss = [psb.tile([P, HID2], F32, tag="psb", name=f"psb2{_t}") for _t in range(GT)]
            for k in range(HID // P):
                wh = wtile("l2", "hi", [k], k, 0, HID2, "wb_hi")
                wl = wtile("l2", "lo", [k], k, 0, HID2, "wb_lo")
                for t in range(GT):
                    mm3(pss[t][:], x1T_hi[:, k, t * P:(t + 1) * P],
                        x1T_lo[:, k, t * P:(t + 1) * P],
                        wh[:, 0, :], wl[:, 0, :], k == 0, k == HID // P - 1)
            for t in range(GT):
                nc.vector.scalar_tensor_tensor(
                    z2[:, t, :], pss[t][:], 1.0, b_l2[:],
                    op0=OP.mult, op1=OP.add)

            x2 = act.tile([P, GT, HID2], F32, tag="xz1", name="x2")
            kwta(z2, x2, kks[1], HID2, g, 2)
            x2T_hi = act.tile([P, HID2 // P, BG], F32R, tag="T_hi1", name="x2T_hi")
            x2T_lo = act.tile([P, HID2 // P, BG], F32R, tag="T_lo1", name="x2T_lo")
            transpose_split(x2, x2T_hi, x2T_lo, HID2)

            # ---- l3 (a): z3T [P, BG] = l3w @ x2 + b  (out=128 rows)
            ps3 = psa.tile([P, BG], F32, tag="psa")
            wh = wtile("l3", "hi", list(range(HID2 // P)), 0, 0, P, "wa_hi")
            wl = wtile("l3", "lo", list(range(HID2 // P)), 0, 0, P, "wa_lo")
            for k in range(HID2 // P):
                mm3(ps3[:], wh[:, k, :], wl[:, k, :],
                    x2T_hi[:, k, :], x2T_lo[:, k, :],
                    k == 0, k == HID2 // P - 1)
            b_l3 = small.tile([P, 1], F32, tag="b_l3")
            nc.sync.dma_start(b_l3[:], d["l3_bcol"][:])
            z3T = act.tile([P, BG], F32, tag="z3T")
            nc.vector.scalar_tensor_tensor(
                z3T[:], ps3[:], 1.0, b_l3[:].to_broadcast([P, BG]),
                op0=OP.mult, op1=OP.add)

            # transpose z3T -> z3 [P, GT, 128]
            z3 = act.tile([P, GT, HEADS], F32, tag="zb", name="z3")
            for t in range(GT):
                pt = pst.tile([P, P], F32, tag="pstT", name="pt")
                nc.tensor.transpose(pt[:], z3T[:, t * P:(t + 1) * P], ident[:])
                nc.any.tensor_copy(z3[:, t, :], pt[:])

            x3 = act.tile([P, GT, HEADS], F32, tag="xz2", name="x3")
            kwta(z3, x3, kks[2], HEADS, g, 3)
            x3T_hi = act.tile([P, 1, BG], F32R, tag="T_hi2", name="x3T_hi")
            x3T_lo = act.tile([P, 1, BG], F32R, tag="T_lo2", name="x3T_lo")
            transpose_split(x3, x3T_hi, x3T_lo, HEADS)

            # ---- l4 (a): outT_g [P, BG]
            ps4 = psa.tile([P, BG], F32, tag="psa")
            wh = wtile("l4", "hi", [0], 0, 0, P, "wa_hi")
            wl = wtile("l4", "lo", [0], 0, 0, P, "wa_lo")
            mm3(ps4[:], wh[:, 0, :], wl[:, 0, :],
                x3T_hi[:, 0, :], x3T_lo[:, 0, :], True, True)
            b_l4 = small.tile([P, 1], F32, tag="b_l4")
            nc.sync.dma_start(b_l4[:], d["l4_bcol"][:])
            og = scratch.tile([P, BG], F32, tag="sc512", name="og")
            nc.vector.scalar_tensor_tensor(
                og[:], ps4[:], 1.0, b_l4[:].to_broadcast([P, BG]),
                op0=OP.mult, op1=OP.add)
            nc.sync.dma_start(outT[:, col0:col0 + BG], og[:])


# ----------------------------------------------------------------------------
# host wrapper
# ----------------------------------------------------------------------------

_CACHE = {}


def _get_program():
    if "nc" not in _CACHE:
        _CACHE["nc"] = build_program()
    return _CACHE["nc"]


def prepare_in_maps(state, task_indicator, cx11_w, cx11_b, cx12_w, cx12_b,
                    cx21_w, cx21_b, cx22_w, cx22_b, cx31_w, cx31_b,
                    cx32_w, cx32_b, l1_w, l1_b, l2_w, l2_b, l3_w, l3_b,
                    l4_w, l4_b):
    state = np.asarray(state, dtype=np.float32)
    task = np.asarray(task_indicator, dtype=np.float32)

    ci = np.concatenate([state, task], axis=1)           # [B, 1028]
    ciT = np.zeros((INP, B), dtype=np.float32)
    ciT[:IN] = ci.T
    ciT_hi = rne12(ciT)
    ciT_lo = (ciT - ciT_hi).astype(np.float32)

    common = {}
    ws = dict(cx11=(cx11_w, cx11_b), cx12=(cx12_w, cx12_b),
              cx21=(cx21_w, cx21_b), cx22=(cx22_w, cx22_b),
              cx31=(cx31_w, cx31_b), cx32=(cx32_w, cx32_b),
              l1=(l1_w, l1_b), l2=(l2_w, l2_b), l3=(l3_w, l3_b),
              l4=(l4_w, l4_b))
    for name, (w, b) in ws.items():
        w = np.asarray(w, dtype=np.float32)
        b = np.asarray(b, dtype=np.float32)
        wT = _pad_k(np.ascontiguousarray(w.T), w.shape[1])
        hi = rne12(wT)
        common[f"{name}_hi"] = hi
        common[f"{name}_lo"] = (wT - hi).astype(np.float32)
        if name in ("cx12", "cx22", "cx32", "l1", "l2"):
            common[f"{name}_brep"] = np.ascontiguousarray(
                np.broadcast_to(b[None, :], (P, b.shape[0]))).astype(np.float32)
        else:
            common[f"{name}_bcol"] = np.ascontiguousarray(
                b.reshape(-1, P).T).astype(np.float32)

    in_maps = []
    for c in range(NCORES):
        m = dict(common)
        sl = slice(c * BC, (c + 1) * BC)
        m["ciT_hi"] = np.ascontiguousarray(ciT_hi[:, sl])
        m["ciT_lo"] = np.ascontiguousarray(ciT_lo[:, sl])
        in_maps.append(m)
    return in_maps


def kernel(**inputs):
    _trace = bool(inputs.pop("_trace", False))
    nc = _get_program()
    in_maps = prepare_in_maps(**inputs)
    res = run_bass_kernel_spmd(nc, in_maps, core_ids=list(range(NCORES)),
                               trace=_trace)
    kernel.last_exec_time_ns = res.exec_time_ns
    out = np.concatenate([r["outT"].T for r in res.results], axis=0)
    return out.astype(np.float32)


kernel.last_exec_time_ns = None



# revision 17
# speedup vs baseline: 2.5132x; 2.5132x over previous
"""Trainium2 Bass kernel for nn_NeuralNetwork_S (kwta / topk_masking) — v2.

Strategy vs v1:
- Native fp32 matmuls (probe: max rel err 1.8e-7, same as v1's 3-term f32r
  split) -> no hi/lo splits anywhere: half the shipped bytes, no host-side
  rne12, no DVE subtract passes.
- Host ships raw per-core row slices of state/task (zero-copy views) and
  cached fp32 w^T; ci transpose happens on device (PE transpose + Pool copy).
- Biases of the 4 IN-facing layers fold into an augmented K=5 tail matmul
  (task^T rows + ones row) x (w_tail rows + bias row) — free on PE since
  matmul cost is N-driven.
- Software-pipelined emission: per group g, phase A (l1 + cx chains) and
  phases B1/B2/B3 (kwta1+l2 / kwta2+l3 / kwta3+l4) are emitted as woven
  generators so group g's kwta bisections (ACT/Pool/DVE) hide under group
  g+1's matmul stream (PE).
- kwta bisection: counts on ACT (sigmoid step w/ 2^100 scale, exact),
  interval ping-pong small ops split DVE/Pool to stay within the 4-deep
  wait stations.
"""

import sys

_TRN = "/opt/trn_rl_repo"
if _TRN not in sys.path:
    sys.path.insert(0, _TRN)

import numpy as np
import concourse.bass as bass
import concourse.mybir as mybir
import concourse.tile as tile
from concourse import bacc
from concourse.bass_utils import run_bass_kernel_spmd
from concourse.masks import make_identity

P = 128
B = 16384
NCORES = 8
BC = B // NCORES          # 2048 rows per core
BG = 512                  # rows per group
NG = BC // BG             # 4 groups
GT = BG // P              # 4 row-tiles per group
IN = 1028
KIN = 8                   # full 128-row k-chunks of the 1024 state features
HID = 1024
HID2 = 512
HEADS = 128

F32 = mybir.dt.float32
U8 = mybir.dt.uint8
I32 = mybir.dt.int32
U32 = mybir.dt.uint32
BF16 = mybir.dt.bfloat16
AF = mybir.ActivationFunctionType
OP = mybir.AluOpType
AX = mybir.AxisListType

SCALE = float(2.0 ** 100)
ITERS = {1024: 12, 512: 12, 128: 10}
THIRD = 1.0 / 3.0

# layer tables ---------------------------------------------------------------
# IN-layers (read ci): (name, out, form); form 'a' = out-on-partitions,
# 'b' = rows-on-partitions
IN_LAYERS = {"cx11": HID, "cx21": HID2, "cx31": HEADS, "l1": HID}
# hidden layers: name -> (k_in, out)
HID_LAYERS = {"cx12": (HID, HID), "cx22": (HID2, HID2), "cx32": (HEADS, HEADS),
              "l2": (HID, HID2), "l3": (HID2, HEADS), "l4": (HEADS, HEADS)}


def build_program():
    nc = bacc.Bacc("TRN2", target_bir_lowering=False, debug=False)
    d = {}

    def din(name, shape, dt=F32):
        d[name] = nc.dram_tensor(name, list(shape), dt, kind="ExternalInput")
        return d[name]

    din("state", [BC, 1024])
    din("task", [BC, 4])
    for name, o in IN_LAYERS.items():
        din(f"{name}_wT", [1024, o])
        din(f"{name}_tail", [5, o])
    for name, (k, o) in HID_LAYERS.items():
        din(f"{name}_wT", [k, o])
    for name in ("cx12", "cx22", "cx32", "l2"):
        din(f"{name}_brep", [P, HID_LAYERS[name][1]])
    for name in ("l3", "l4"):
        din(f"{name}_bcol", [P, 1])

    outT = nc.dram_tensor("outT", [P, BC], F32, kind="ExternalOutput")

    with tile.TileContext(nc) as tc:
        _emit(tc, nc, d, outT)
    nc.compile()
    return nc


def _emit(tc, nc, d, outT):
    import contextlib

    ctx = contextlib.ExitStack()
    with ctx:
        big = ctx.enter_context(tc.tile_pool(name="big", bufs=1))
        dbuf = ctx.enter_context(tc.tile_pool(name="dbuf", bufs=1))
        shared = ctx.enter_context(tc.tile_pool(name="shared", bufs=2))
        wts = ctx.enter_context(tc.tile_pool(name="wts", bufs=2))
        cons = ctx.enter_context(tc.tile_pool(name="cons", bufs=1))
        small = ctx.enter_context(tc.tile_pool(name="small", bufs=4))
        scr = ctx.enter_context(tc.tile_pool(name="scr", bufs=1))
        psb = ctx.enter_context(tc.tile_pool(name="psb", bufs=1, space="PSUM"))
        psa = ctx.enter_context(tc.tile_pool(name="psa", bufs=2, space="PSUM"))
        pst = ctx.enter_context(tc.tile_pool(name="pst", bufs=2, space="PSUM"))

        # constants ----------------------------------------------------------
        ident = cons.tile([P, P], F32, tag="ident")
        make_identity(nc, ident[:])
        negbig = cons.tile([P, 1], F32, tag="negbig")
        nc.vector.memset(negbig[:], -1.0e30)
        iota8 = cons.tile([P, 8], F32, tag="iota8")
        iota8u = small.tile([P, 8], U32, tag="iota8u")
        nc.gpsimd.iota(iota8u[:], pattern=[[1, 8]], base=0, channel_multiplier=0)
        nc.vector.tensor_copy(iota8[:], iota8u[:])
        zbias = cons.tile([P, 1], F32, tag="zbias")
        nc.vector.memset(zbias[:], 0.0)

        # resident weights: tails + breps + bcols -----------------------------
        tails = {}
        for name, o in IN_LAYERS.items():
            t = cons.tile([5, o], F32, tag=f"tail_{name}")
            nc.sync.dma_start(t[:], d[f"{name}_tail"][:])
            tails[name] = t
        breps = {}
        for name in ("cx12", "cx22", "cx32", "l2"):
            t = cons.tile([P, HID_LAYERS[name][1]], F32, tag=f"brep_{name}")
            nc.sync.dma_start(t[:], d[f"{name}_brep"][:])
            breps[name] = t
        bcols = {}
        for name in ("l3", "l4"):
            t = cons.tile([P, 1], F32, tag=f"bcol_{name}")
            nc.sync.dma_start(t[:], d[f"{name}_bcol"][:])
            bcols[name] = t

        state_r = d["state"].rearrange("(n p) f -> p n f", p=P)   # [P,16,1024]
        task_r = d["task"].rearrange("(n p) f -> p n f", p=P)     # [P,16,4]

        def wslab_b(name, k, n0, nw):
            """(b)-form moving slab [P, 1, nw] from wT rows [k*128, +128)."""
            t = wts.tile([P, 1, nw], F32, tag="wb")
            src = d[f"{name}_wT"].rearrange("(c p) o -> p c o", p=P)
            nc.sync.dma_start(t[:], src[:, k:k + 1, n0:n0 + nw])
            return t

        def wslab_a(name, k0, kc, m0, mw):
            """(a)-form stationary slab [P, kc<=4, mw] (k-chunks k0..k0+kc)."""
            t = wts.tile([P, kc, mw], F32, tag="wa")
            src = d[f"{name}_wT"].rearrange("(c p) o -> p c o", p=P)
            nc.sync.dma_start(t[:], src[:, k0:k0 + kc, m0:m0 + mw])
            return t

        # ---------------- phase A: ci transpose, l1, cx chains ---------------
        def phase_a(g, st):
            col0 = g * BG
            ciT = shared.tile([P, KIN, BG], F32, tag="big16", name="ciT")
            taskT = big.tile([5, BG], F32, tag="taskT")
            tTASK = small.tile([P, GT, 5], F32, tag="tTASK")
            nc.sync.dma_start(tTASK[:, :, 0:4], task_r[:, g * GT:(g + 1) * GT, :])
            nc.gpsimd.memset(tTASK[:, :, 4:5], 1.0)
            yield
            # transpose ci into [feature-part, row] layout
            for t in range(GT):
                sROW = dbuf.tile([P, 1024], F32, tag="sROW")
                nc.sync.dma_start(sROW[:], state_r[:, g * GT + t, :])
                for c0 in (0, 4):
                    ps = pst.tile([P, 4 * P], F32, tag="pst")
                    for c in range(c0, c0 + 4):
                        nc.tensor.transpose(
                            ps[:, (c - c0) * P:(c - c0 + 1) * P],
                            sROW[:, c * P:(c + 1) * P], ident[:])
                    dst = ciT[:, c0:c0 + 4, t * P:(t + 1) * P]
                    src = ps[:].rearrange("p (c q) -> p c q", q=P)
                    nc.vector.tensor_copy(dst, src)
                    yield
                pt = pst.tile([P, 4 * P], F32, tag="pst")
                nc.tensor.transpose(pt[0:5, 0:P], tTASK[:, t, :], ident[:])
                nc.vector.tensor_copy(taskT[0:5, t * P:(t + 1) * P],
                                      pt[0:5, 0:P])
                yield

            # ---- l1 (b): z1 [P, GT, 1024]
            z1 = shared.tile([P, GT, HID], F32, tag="z1", name="z1")
            st["z1"] = z1
            for n0 in range(0, HID, 512):
                ps = psb.tile([P, GT, 512], F32, tag="psb")
                for k in range(KIN):
                    wb = wslab_b("l1", k, n0, 512)
                    for t in range(GT):
                        nc.tensor.matmul(
                            ps[:, t, :], ciT[:, k, t * P:(t + 1) * P],
                            wb[:, 0, :], start=(k == 0), stop=False)
                    yield
                for t in range(GT):
                    nc.tensor.matmul(
                        ps[:, t, :], taskT[0:5, t * P:(t + 1) * P],
                        tails["l1"][0:5, n0:n0 + 512], start=False, stop=True)
                yield
                for t in range(GT):
                    nc.vector.tensor_copy(z1[:, t, n0:n0 + 512], ps[:, t, :])
                yield

            # ---- cx chains -> kk0/kk1/kk2
            for cn, (pre, post, hidn, mch) in enumerate(
                [("cx11", "cx12", HID, 8), ("cx21", "cx22", HID2, 4),
                 ("cx31", "cx32", HEADS, 1)]
            ):
                kc_pre = KIN
                httag = {0: "hx1", 1: "hx2", 2: "hx3"}[cn]
                hT = shared.tile([P, mch, BG], F32, tag=httag, name=f"hT{cn}")
                for m in range(mch):
                    ps = psa.tile([P, BG], F32, tag="psa")
                    for k0 in range(0, kc_pre, 4):
                        wa = wslab_a(pre, k0, 4, m * P, P)
                        for k in range(k0, k0 + 4):
                            nc.tensor.matmul(ps[:], wa[:, k - k0, :],
                                             ciT[:, k, :],
                                             start=(k == 0), stop=False)
                    nc.tensor.matmul(ps[:], tails[pre][0:5, m * P:(m + 1) * P],
                                     taskT[0:5, :], start=False, stop=True)
                    nc.scalar.activation(hT[:, m, :], ps[:], AF.Tanh,
                                         bias=zbias[:], scale=1.0)
                    yield

                # second layer (b) + incremental argmax
                kk = small.tile([P, GT], F32, tag=f"kk{cn}", name="kk")
                st[f"kk{cn}"] = kk
                kin2, out2 = HID_LAYERS[post]
                bestm = small.tile([P, GT], F32, tag="bestm")
                kkA = small.tile([P, GT], F32, tag="kkA")
                n0s = list(range(0, out2, 512))
                for ci_, n0 in enumerate(n0s):
                    nw = min(512, out2)
                    ps = psb.tile([P, GT, 512], F32, tag="psb")
                    for k in range(mch):
                        wb = wslab_b(post, k, n0, nw)
                        for t in range(GT):
                            nc.tensor.matmul(
                                ps[:, t, 0:nw], hT[:, k, t * P:(t + 1) * P],
                                wb[:, 0, :], start=(k == 0), stop=(k == mch - 1))
                        yield
                    m8 = small.tile([P, 8], F32, tag="am8")
                    idx = small.tile([P, 8], U32, tag="aidx")
                    idxf = small.tile([P, 8], F32, tag="aidxf")
                    for t in range(GT):
                        zcx = big.tile([P, 512], F32, tag="zcx", name="zcx")
                        nc.vector.scalar_tensor_tensor(
                            zcx[:, 0:nw], ps[:, t, 0:nw], 1.0,
                            breps[post][:, n0:n0 + nw], op0=OP.mult, op1=OP.add)
                        nc.vector.max(out=m8[:], in_=zcx[:, 0:nw])
                        nc.vector.max_index(idx[:], m8[:], zcx[:, 0:nw])
                        nc.vector.tensor_copy(idxf[:, 0:1], idx[:, 0:1])
                        if ci_ == 0 and len(n0s) == 1:
                            nc.vector.tensor_copy(kk[:, t:t + 1], idxf[:, 0:1])
                        elif ci_ == 0:
                            nc.vector.tensor_copy(kkA[:, t:t + 1], idxf[:, 0:1])
                            nc.vector.tensor_copy(bestm[:, t:t + 1],
                                                  m8[:, 0:1])
                        else:
                            gtu = small.tile([P, 1], U8, tag="agt")
                            nc.vector.tensor_tensor(
                                gtu[:], m8[:, 0:1], bestm[:, t:t + 1],
                                op=OP.is_gt)
                            i2 = small.tile([P, 1], F32, tag="ai2")
                            nc.vector.tensor_scalar(
                                i2[:], idxf[:, 0:1], float(n0), None,
                                op0=OP.add)
                            nc.vector.select(kk[:, t:t + 1], gtu[:], i2[:],
                                             kkA[:, t:t + 1])
                        yield

        # ---------------- kwta bisection ------------------------------------
        def kwta(zg, xg, kk, n):
            I = ITERS[n]
            loA = small.tile([P, GT], F32, tag="kwloA")
            loB = small.tile([P, GT], F32, tag="kwloB")
            hiA = small.tile([P, GT], F32, tag="kwhiA")
            hiB = small.tile([P, GT], F32, tag="kwhiB")
            chA = small.tile([P, GT], F32, tag="kwchA")
            chB = small.tile([P, GT], F32, tag="kwchB")
            cnt = small.tile([P, GT], F32, tag="kwcnt")
            kp1 = small.tile([P, GT], F32, tag="kwkp1")
            msum = small.tile([P, GT], F32, tag="kwmsum")
            mid = small.tile([P, GT], F32, tag="kwmid")
            nbias = small.tile([P, GT], F32, tag="kwnb")
            mn = small.tile([P, GT], F32, tag="kwmn")
            selu = small.tile([P, GT], U8, tag="kwselu")
            trash = scr.tile([P, n], BF16, tag=f"kwA{n}", name="trash")

            nc.gpsimd.tensor_scalar(kp1[:], kk[:], 1.0, None, op0=OP.add)
            nc.gpsimd.memset(chA[:], 0.0)
            for t in range(GT):
                nc.vector.reduce_max(hiA[:, t:t + 1], zg[:, t, :], axis=AX.X)
                nc.vector.tensor_reduce(out=mn[:, t:t + 1], in_=zg[:, t, :],
                                        op=OP.min, axis=AX.X)
            nc.gpsimd.tensor_scalar(loA[:], mn[:], 1.0, None, op0=OP.subtract)
            yield

            lo, hi, ch = loA, hiA, chA
            lon, hin, chn = loB, hiB, chB
            for it in range(I):
                nc.gpsimd.tensor_tensor(msum[:], lo[:], hi[:], op=OP.add)
                nc.gpsimd.tensor_scalar(mid[:], msum[:], 0.5, None,
                                        op0=OP.mult)
                nc.gpsimd.tensor_scalar(nbias[:], mid[:], -SCALE, None,
                                        op0=OP.mult)
                for t in range(GT):
                    nc.scalar.activation(
                        trash[:], zg[:, t, :], AF.Sigmoid,
                        bias=nbias[:, t:t + 1], scale=SCALE,
                        accum_out=cnt[:, t:t + 1])
                nc.vector.tensor_tensor(selu[:], cnt[:], kp1[:], op=OP.is_ge)
                nc.vector.select(lon[:], selu[:], mid[:], lo[:])
                nc.vector.select(hin[:], selu[:], hi[:], mid[:])
                nc.vector.select(chn[:], selu[:], ch[:], cnt[:])
                lo, lon = lon, lo
                hi, hin = hin, hi
                ch, chn = chn, ch
                yield

            chii = small.tile([P, GT], I32, tag="kwchii")
            nc.vector.tensor_scalar(chn[:], ch[:], 0.25, None, op0=OP.subtract)
            nc.vector.tensor_copy(chii[:], chn[:])
            nc.vector.tensor_copy(ch[:], chii[:])
            rm1 = small.tile([P, GT], F32, tag="kwrm1")
            nc.vector.tensor_tensor(rm1[:], kk[:], ch[:], op=OP.subtract)
            yield

            for t in range(GT):
                m1 = scr.tile([P, n], F32, tag=f"kwA{n}", name="m1")
                gu8 = scr.tile([P, n], U8, tag=f"kwgu{n}", name="gu8")
                msk = scr.tile([P, n], F32, tag=f"kwmsk{n}", name="msk")
                nc.gpsimd.tensor_scalar(m1[:], zg[:, t, :], lo[:, t:t + 1],
                                        None, op0=OP.max)
                nc.vector.tensor_scalar(gu8[:], zg[:, t, :], hi[:, t:t + 1],
                                        None, op0=OP.is_gt)
                nc.vector.select(msk[:], gu8[:], negbig[:].to_broadcast([P, n]),
                                 m1[:])
                m8 = small.tile([P, 8], F32, tag="kwm8")
                nc.vector.max(out=m8[:], in_=msk[:])
                eq = small.tile([P, 8], F32, tag="kweq")
                nc.vector.tensor_scalar(eq[:], iota8[:], rm1[:, t:t + 1],
                                        None, op0=OP.is_equal)
                pr = small.tile([P, 8], F32, tag="kwpr")
                nc.vector.tensor_tensor(pr[:], eq[:], m8[:], op=OP.mult)
                u = small.tile([P, 1], F32, tag="kwu")
                nc.vector.reduce_sum(u[:], pr[:], axis=AX.X)
                yield
                geu = scr.tile([P, n], U8, tag=f"kwgu{n}", name="geu")
                nc.vector.tensor_scalar(geu[:], zg[:, t, :], u[:], None,
                                        op0=OP.is_gt)
                zth = scr.tile([P, n], F32, tag=f"kwA{n}", name="zth")
                nc.gpsimd.tensor_scalar(zth[:], zg[:, t, :], THIRD, None,
                                        op0=OP.mult)
                nc.vector.select(xg[:, t, :], geu[:], zg[:, t, :], zth[:])
                yield

        # transpose [P, GT, n] -> xT [P, n//P, BG]
        def transpose_x(xg, xT, n):
            nch = n // P
            for t in range(GT):
                for c0 in range(0, nch, 4):
                    cw = min(4, nch - c0)
                    ps = pst.tile([P, 4 * P], F32, tag="pst")
                    for c in range(c0, c0 + cw):
                        nc.tensor.transpose(
                            ps[:, (c - c0) * P:(c - c0 + 1) * P],
                            xg[:, t, c * P:(c + 1) * P], ident[:])
                    dst = xT[:, c0:c0 + cw, t * P:(t + 1) * P]
                    src = ps[:, 0:cw * P].rearrange("p (c q) -> p c q", q=P)
                    nc.vector.tensor_copy(dst, src)
                    yield

        # ---------------- phase B1: kwta1, x1T, l2 ---------------------------
        def phase_b1(g, st):
            x1 = shared.tile([P, GT, HID], F32, tag="big16", name="x1")
            yield from kwta(st["z1"], x1, st["kk0"], HID)
            x1T = shared.tile([P, HID // P, BG], F32, tag="hx1", name="x1T")
            yield from transpose_x(x1, x1T, HID)
            z2 = shared.tile([P, GT, HID2], F32, tag="z2")
            st["z2"] = z2
            ps = psb.tile([P, GT, 512], F32, tag="psb")
            for k in range(HID // P):
                wb = wslab_b("l2", k, 0, HID2)
                for t in range(GT):
                    nc.tensor.matmul(
                        ps[:, t, :], x1T[:, k, t * P:(t + 1) * P],
                        wb[:, 0, :], start=(k == 0), stop=(k == HID // P - 1))
                yield
            for t in range(GT):
                nc.vector.scalar_tensor_tensor(
                    z2[:, t, :], ps[:, t, :], 1.0, breps["l2"][:],
                    op0=OP.mult, op1=OP.add)
            yield

        # ---------------- phase B2: kwta2, x2T, l3 ---------------------------
        def phase_b2(g, st):
            x2 = big.tile([P, GT, HID2], F32, tag="x2")
            yield from kwta(st["z2"], x2, st["kk1"], HID2)
            x2T = shared.tile([P, HID2 // P, BG], F32, tag="hx2", name="x2T")
            yield from transpose_x(x2, x2T, HID2)
            ps3 = psa.tile([P, BG], F32, tag="psa")
            wa = wslab_a("l3", 0, HID2 // P, 0, P)
            for k in range(HID2 // P):
                nc.tensor.matmul(ps3[:], wa[:, k, :], x2T[:, k, :],
                                 start=(k == 0), stop=(k == HID2 // P - 1))
            z3T = big.tile([P, BG], F32, tag="zot", name="z3T")
            nc.vector.scalar_tensor_tensor(
                z3T[:], ps3[:], 1.0, bcols["l3"][:].to_broadcast([P, BG]),
                op0=OP.mult, op1=OP.add)
            yield
            z3 = shared.tile([P, GT, HEADS], F32, tag="z3")
            st["z3"] = z3
            for t in range(GT):
                pt = pst.tile([P, 4 * P], F32, tag="pst")
                nc.tensor.transpose(pt[:, 0:P], z3T[:, t * P:(t + 1) * P],
                                    ident[:])
                nc.vector.tensor_copy(z3[:, t, :], pt[:, 0:P])
            yield

        # ---------------- phase B3: kwta3, x3T, l4, out ----------------------
        def phase_b3(g, st):
            col0 = g * BG
            x3 = big.tile([P, GT, HEADS], F32, tag="x3")
            yield from kwta(st["z3"], x3, st["kk2"], HEADS)
            x3T = shared.tile([P, 1, BG], F32, tag="hx3", name="x3T")
            yield from transpose_x(x3, x3T, HEADS)
            ps4 = psa.tile([P, BG], F32, tag="psa")
            wa = wslab_a("l4", 0, 1, 0, P)
            nc.tensor.matmul(ps4[:], wa[:, 0, :], x3T[:, 0, :],
                             start=True, stop=True)
            og = big.tile([P, BG], F32, tag="zot", name="og")
            nc.vector.scalar_tensor_tensor(
                og[:], ps4[:], 1.0, bcols["l4"][:].to_broadcast([P, BG]),
                op0=OP.mult, op1=OP.add)
            nc.sync.dma_start(outT[:, col0:col0 + BG], og[:])
            yield

        # ---------------- weave ------------------------------------------
        sts = [dict() for _ in range(NG)]

        def weave(gens):
            active = list(gens)
            while active:
                keep = []
                for it in active:
                    try:
                        next(it)
                        keep.append(it)
                    except StopIteration:
                        pass
                active = keep

        slots = [
            [phase_a(0, sts[0])],
            [phase_a(1, sts[1])],
            [phase_a(2, sts[2]), phase_b1(0, sts[0])],
            [phase_a(3, sts[3]), phase_b2(0, sts[0]), phase_b1(1, sts[1])],
            [phase_b3(0, sts[0]), phase_b2(1, sts[1]), phase_b1(2, sts[2])],
            [phase_b3(1, sts[1]), phase_b2(2, sts[2]), phase_b1(3, sts[3])],
            [phase_b3(2, sts[2]), phase_b2(3, sts[3])],
            [phase_b3(3, sts[3])],
        ]
        for s in slots:
            weave(s)


# ----------------------------------------------------------------------------
# host wrapper
# ----------------------------------------------------------------------------

_CACHE = {}


def _get_program():
    if "nc" not in _CACHE:
        _CACHE["nc"] = build_program()
    return _CACHE["nc"]


def _fingerprint(arrs):
    out = []
    for a in arrs:
        out.append((id(a), a.shape, a.dtype.str,
                    float(a.flat[0]), float(a.flat[-1])))
    return tuple(out)


def _prep_weights(ws):
    """ws: dict name -> (w, b). Returns the replicated input map (cached)."""
    arrs = [a for pair in ws.values() for a in pair]
    key = _fingerprint(arrs)
    hit = _CACHE.get("wkey")
    if hit == key:
        return _CACHE["wmap"]
    m = {}
    for name, (w, b) in ws.items():
        w = np.asarray(w, dtype=np.float32)
        b = np.asarray(b, dtype=np.float32)
        if name in IN_LAYERS:
            m[f"{name}_wT"] = np.ascontiguousarray(w[:, :1024].T)
            m[f"{name}_tail"] = np.ascontiguousarray(
                np.vstack([w[:, 1024:1028].T, b[None, :]]))
        else:
            m[f"{name}_wT"] = np.ascontiguousarray(w.T)
            if name in ("l3", "l4"):
                m[f"{name}_bcol"] = np.ascontiguousarray(
                    np.broadcast_to(b[:, None], (P, 1)))
            else:
                m[f"{name}_brep"] = np.ascontiguousarray(
                    np.broadcast_to(b[None, :], (P, b.shape[0])))
    _CACHE["wkey"] = key
    _CACHE["wmap"] = m
    return m


def kernel(**inputs):
    _trace = bool(inputs.pop("_trace", False))
    nc = _get_program()
    state = np.asarray(inputs["state"], dtype=np.float32)
    task = np.asarray(inputs["task_indicator"], dtype=np.float32)
    ws = {n: (inputs[f"{n}_w"], inputs[f"{n}_b"])
          for n in list(IN_LAYERS) + list(HID_LAYERS)}
    common = _prep_weights(ws)
    in_maps = []
    for c in range(NCORES):
        m = dict(common)
        m["state"] = state[c * BC:(c + 1) * BC]
        m["task"] = task[c * BC:(c + 1) * BC]
        in_maps.append(m)
    res = run_bass_kernel_spmd(nc, in_maps, core_ids=list(range(NCORES)),
                               trace=_trace)
    kernel.last_exec_time_ns = res.exec_time_ns
    out = np.concatenate([r["outT"].T for r in res.results], axis=0)
    return np.ascontiguousarray(out, dtype=np.float32)


kernel.last_exec_time_ns = None


# revision 26
# speedup vs baseline: 2.7997x; 1.1140x over previous
"""Trainium2 Bass kernel for nn_NeuralNetwork_S (kwta / topk_masking) — v2.

Strategy vs v1:
- Native fp32 matmuls (probe: max rel err 1.8e-7, same as v1's 3-term f32r
  split) -> no hi/lo splits anywhere: half the shipped bytes, no host-side
  rne12, no DVE subtract passes.
- Host ships raw per-core row slices of state/task (zero-copy views) and
  cached fp32 w^T; ci transpose happens on device (PE transpose + Pool copy).
- Biases of the 4 IN-facing layers fold into an augmented K=5 tail matmul
  (task^T rows + ones row) x (w_tail rows + bias row) — free on PE since
  matmul cost is N-driven.
- Software-pipelined emission: per group g, phase A (l1 + cx chains) and
  phases B1/B2/B3 (kwta1+l2 / kwta2+l3 / kwta3+l4) are emitted as woven
  generators so group g's kwta bisections (ACT/Pool/DVE) hide under group
  g+1's matmul stream (PE).
- kwta bisection: counts on ACT (sigmoid step w/ 2^100 scale, exact),
  interval ping-pong small ops split DVE/Pool to stay within the 4-deep
  wait stations.
"""

import sys

_TRN = "/opt/trn_rl_repo"
if _TRN not in sys.path:
    sys.path.insert(0, _TRN)

import numpy as np
import concourse.bass as bass
import concourse.mybir as mybir
import concourse.tile as tile
from concourse import bacc
from concourse.bass_utils import run_bass_kernel_spmd
from concourse.masks import make_identity

P = 128
B = 16384
NCORES = 8
BC = B // NCORES          # 2048 rows per core
BG = 512                  # rows per group
NG = BC // BG             # 4 groups
GT = BG // P              # 4 row-tiles per group
IN = 1028
KIN = 8                   # full 128-row k-chunks of the 1024 state features
HID = 1024
HID2 = 512
HEADS = 128

F32 = mybir.dt.float32
U8 = mybir.dt.uint8
I32 = mybir.dt.int32
U32 = mybir.dt.uint32
BF16 = mybir.dt.bfloat16
AF = mybir.ActivationFunctionType
OP = mybir.AluOpType
AX = mybir.AxisListType

SCALE = float(2.0 ** 100)
ITERS = {1024: 12, 512: 12, 128: 10}
THIRD = 1.0 / 3.0

# layer tables ---------------------------------------------------------------
# IN-layers (read ci): (name, out, form); form 'a' = out-on-partitions,
# 'b' = rows-on-partitions
IN_LAYERS = {"cx11": HID, "cx21": HID2, "cx31": HEADS, "l1": HID}
# hidden layers: name -> (k_in, out)
HID_LAYERS = {"cx12": (HID, HID), "cx22": (HID2, HID2), "cx32": (HEADS, HEADS),
              "l2": (HID, HID2), "l3": (HID2, HEADS), "l4": (HEADS, HEADS)}


def build_program():
    nc = bacc.Bacc("TRN2", target_bir_lowering=False, debug=False)
    d = {}

    def din(name, shape, dt=F32):
        d[name] = nc.dram_tensor(name, list(shape), dt, kind="ExternalInput")
        return d[name]

    din("state", [BC, 1024])
    din("task", [BC, 4])
    for name, o in IN_LAYERS.items():
        din(f"{name}_wT", [1024, o])
        din(f"{name}_tail", [5, o])
    for name, (k, o) in HID_LAYERS.items():
        din(f"{name}_wT", [k, o])
    for name in ("cx12", "cx22", "cx32", "l2"):
        din(f"{name}_brep", [P, HID_LAYERS[name][1]])
    for name in ("l3", "l4"):
        din(f"{name}_bcol", [P, 1])

    outT = nc.dram_tensor("outT", [P, BC], F32, kind="ExternalOutput")

    with tile.TileContext(nc) as tc:
        _emit(tc, nc, d, outT)
    nc.compile()
    return nc


def _emit(tc, nc, d, outT):
    import contextlib

    ctx = contextlib.ExitStack()
    with ctx:
        big = ctx.enter_context(tc.tile_pool(name="big", bufs=1))
        dbuf = ctx.enter_context(tc.tile_pool(name="dbuf", bufs=1))
        shared = ctx.enter_context(tc.tile_pool(name="shared", bufs=2))
        wts = ctx.enter_context(tc.tile_pool(name="wts", bufs=2))
        cons = ctx.enter_context(tc.tile_pool(name="cons", bufs=1))
        small = ctx.enter_context(tc.tile_pool(name="small", bufs=4))
        scr = ctx.enter_context(tc.tile_pool(name="scr", bufs=1))
        psb = ctx.enter_context(tc.tile_pool(name="psb", bufs=1, space="PSUM"))
        psa = ctx.enter_context(tc.tile_pool(name="psa", bufs=2, space="PSUM"))
        pst = ctx.enter_context(tc.tile_pool(name="pst", bufs=2, space="PSUM"))

        # constants ----------------------------------------------------------
        ident = cons.tile([P, P], F32, tag="ident")
        make_identity(nc, ident[:])
        negbig = cons.tile([P, 1], F32, tag="negbig")
        nc.vector.memset(negbig[:], -1.0e30)
        iota8 = cons.tile([P, 8], F32, tag="iota8")
        iota8u = small.tile([P, 8], U32, tag="iota8u")
        nc.gpsimd.iota(iota8u[:], pattern=[[1, 8]], base=0, channel_multiplier=0)
        nc.vector.tensor_copy(iota8[:], iota8u[:])
        zbias = cons.tile([P, 1], F32, tag="zbias")
        nc.vector.memset(zbias[:], 0.0)

        # resident weights: tails + breps + bcols -----------------------------
        tails = {}
        for name, o in IN_LAYERS.items():
            t = cons.tile([5, o], F32, tag=f"tail_{name}")
            nc.sync.dma_start(t[:], d[f"{name}_tail"][:])
            tails[name] = t
        breps = {}
        for name in ("cx12", "cx22", "cx32", "l2"):
            t = cons.tile([P, HID_LAYERS[name][1]], F32, tag=f"brep_{name}")
            nc.sync.dma_start(t[:], d[f"{name}_brep"][:])
            breps[name] = t
        bcols = {}
        for name in ("l3", "l4"):
            t = cons.tile([P, 1], F32, tag=f"bcol_{name}")
            nc.sync.dma_start(t[:], d[f"{name}_bcol"][:])
            bcols[name] = t

        state_r = d["state"].rearrange("(n p) f -> p n f", p=P)   # [P,16,1024]
        task_r = d["task"].rearrange("(n p) f -> p n f", p=P)     # [P,16,4]

        def wslab_b(name, k, n0, nw):
            """(b)-form moving slab [P, 1, nw] from wT rows [k*128, +128)."""
            t = wts.tile([P, 1, nw], F32, tag="wb")
            src = d[f"{name}_wT"].rearrange("(c p) o -> p c o", p=P)
            nc.sync.dma_start(t[:], src[:, k:k + 1, n0:n0 + nw])
            return t

        def wslab_a(name, k0, kc, m0, mw):
            """(a)-form stationary slab [P, kc<=4, mw] (k-chunks k0..k0+kc)."""
            t = wts.tile([P, kc, mw], F32, tag="wa")
            src = d[f"{name}_wT"].rearrange("(c p) o -> p c o", p=P)
            nc.sync.dma_start(t[:], src[:, k0:k0 + kc, m0:m0 + mw])
            return t

        # ---------------- phase A1: ci transpose, l1, cx1 chain --------------
        def phase_a1(g, st):
            col0 = g * BG
            ciT = shared.tile([P, KIN, BG], F32, tag="big16", name="ciT")
            st["ciT"] = ciT
            taskT = big.tile([5, BG], F32, tag="taskT")
            st["taskT"] = taskT
            tTASK = small.tile([P, GT, 5], F32, tag="tTASK")
            nc.sync.dma_start(tTASK[:, :, 0:4], task_r[:, g * GT:(g + 1) * GT, :])
            nc.gpsimd.memset(tTASK[:, :, 4:5], 1.0)
            yield
            # transpose ci into [feature-part, row] layout
            for t in range(GT):
                sROW = dbuf.tile([P, 1024], F32, tag="sROW")
                nc.sync.dma_start(sROW[:], state_r[:, g * GT + t, :])
                for c0 in (0, 4):
                    ps = pst.tile([P, 4 * P], F32, tag="pst")
                    for c in range(c0, c0 + 4):
                        nc.tensor.transpose(
                            ps[:, (c - c0) * P:(c - c0 + 1) * P],
                            sROW[:, c * P:(c + 1) * P], ident[:])
                    dst = ciT[:, c0:c0 + 4, t * P:(t + 1) * P]
                    src = ps[:].rearrange("p (c q) -> p c q", q=P)
                    nc.vector.tensor_copy(dst, src)
                    yield
                pt = pst.tile([P, 4 * P], F32, tag="pst")
                nc.tensor.transpose(pt[0:5, 0:P], tTASK[:, t, :], ident[:])
                nc.vector.tensor_copy(taskT[0:5, t * P:(t + 1) * P],
                                      pt[0:5, 0:P])
                yield

            # ---- l1 (b): z1 [P, GT, 1024]
            z1 = shared.tile([P, GT, HID], F32, tag="z1", name="z1")
            st["z1"] = z1
            for n0 in range(0, HID, 512):
                ps = psb.tile([P, GT, 512], F32, tag="psb")
                for k in range(KIN):
                    wb = wslab_b("l1", k, n0, 512)
                    for t in range(GT):
                        nc.tensor.matmul(
                            ps[:, t, :], ciT[:, k, t * P:(t + 1) * P],
                            wb[:, 0, :], start=(k == 0), stop=False)
                    yield
                for t in range(GT):
                    nc.tensor.matmul(
                        ps[:, t, :], taskT[0:5, t * P:(t + 1) * P],
                        tails["l1"][0:5, n0:n0 + 512], start=False, stop=True)
                yield
                for t in range(GT):
                    nc.vector.tensor_copy(z1[:, t, n0:n0 + 512], ps[:, t, :])
                yield

            # ---- cx1 chain -> kk0
            yield from cx_chain(g, st, 0)

        # ---------------- phase A2: cx2/cx3 chains ---------------------------
        def phase_a2(g, st):
            yield from cx_chain(g, st, 1)
            yield from cx_chain(g, st, 2)

        CX_DEFS = [("cx11", "cx12", HID, 8), ("cx21", "cx22", HID2, 4),
                   ("cx31", "cx32", HEADS, 1)]

        def cx_chain(g, st, cn):
            ciT = st["ciT"]
            taskT = st["taskT"]
            if True:
                pre, post, hidn, mch = CX_DEFS[cn]
                kc_pre = KIN
                httag = {0: "hx1", 1: "hx2", 2: "hx3"}[cn]
                hT = shared.tile([P, mch, BG], F32, tag=httag, name=f"hT{cn}")
                for m in range(mch):
                    ps = psa.tile([P, BG], F32, tag="psa")
                    for k0 in range(0, kc_pre, 4):
                        wa = wslab_a(pre, k0, 4, m * P, P)
                        for k in range(k0, k0 + 4):
                            nc.tensor.matmul(ps[:], wa[:, k - k0, :],
                                             ciT[:, k, :],
                                             start=(k == 0), stop=False)
                    nc.tensor.matmul(ps[:], tails[pre][0:5, m * P:(m + 1) * P],
                                     taskT[0:5, :], start=False, stop=True)
                    nc.scalar.activation(hT[:, m, :], ps[:], AF.Tanh,
                                         bias=zbias[:], scale=1.0)
                    yield

                # second layer (b) + incremental argmax
                kk = small.tile([P, GT], F32, tag=f"kk{cn}", name="kk")
                st[f"kk{cn}"] = kk
                kin2, out2 = HID_LAYERS[post]
                bestm = small.tile([P, GT], F32, tag="bestm")
                kkA = small.tile([P, GT], F32, tag="kkA")
                n0s = list(range(0, out2, 512))
                for ci_, n0 in enumerate(n0s):
                    nw = min(512, out2)
                    ps = psb.tile([P, GT, 512], F32, tag="psb")
                    for k in range(mch):
                        wb = wslab_b(post, k, n0, nw)
                        for t in range(GT):
                            nc.tensor.matmul(
                                ps[:, t, 0:nw], hT[:, k, t * P:(t + 1) * P],
                                wb[:, 0, :], start=(k == 0), stop=(k == mch - 1))
                        yield
                    m8 = small.tile([P, 8], F32, tag="am8")
                    idx = small.tile([P, 8], U32, tag="aidx")
                    idxf = small.tile([P, 8], F32, tag="aidxf")
                    for t in range(GT):
                        zcx = big.tile([P, 512], F32, tag="zcx", name="zcx")
                        nc.vector.scalar_tensor_tensor(
                            zcx[:, 0:nw], ps[:, t, 0:nw], 1.0,
                            breps[post][:, n0:n0 + nw], op0=OP.mult, op1=OP.add)
                        nc.vector.max(out=m8[:], in_=zcx[:, 0:nw])
                        nc.vector.max_index(idx[:], m8[:], zcx[:, 0:nw])
                        nc.vector.tensor_copy(idxf[:, 0:1], idx[:, 0:1])
                        if ci_ == 0 and len(n0s) == 1:
                            nc.vector.tensor_copy(kk[:, t:t + 1], idxf[:, 0:1])
                        elif ci_ == 0:
                            nc.vector.tensor_copy(kkA[:, t:t + 1], idxf[:, 0:1])
                            nc.vector.tensor_copy(bestm[:, t:t + 1],
                                                  m8[:, 0:1])
                        else:
                            gtu = small.tile([P, 1], U8, tag="agt")
                            nc.vector.tensor_tensor(
                                gtu[:], m8[:, 0:1], bestm[:, t:t + 1],
                                op=OP.is_gt)
                            i2 = small.tile([P, 1], F32, tag="ai2")
                            nc.vector.tensor_scalar(
                                i2[:], idxf[:, 0:1], float(n0), None,
                                op0=OP.add)
                            nc.vector.select(kk[:, t:t + 1], gtu[:], i2[:],
                                             kkA[:, t:t + 1])
                        yield

        # ---------------- kwta bisection ------------------------------------
        def kwta(zg, xg, kk, n):
            I = ITERS[n]
            loA = small.tile([P, GT], F32, tag="kwloA")
            loB = small.tile([P, GT], F32, tag="kwloB")
            hiA = small.tile([P, GT], F32, tag="kwhiA")
            hiB = small.tile([P, GT], F32, tag="kwhiB")
            chA = small.tile([P, GT], F32, tag="kwchA")
            chB = small.tile([P, GT], F32, tag="kwchB")
            cnt = small.tile([P, GT], F32, tag="kwcnt")
            kp1 = small.tile([P, GT], F32, tag="kwkp1")
            msum = small.tile([P, GT], F32, tag="kwmsum")
            mid = small.tile([P, GT], F32, tag="kwmid")
            nbias = small.tile([P, GT], F32, tag="kwnb")
            mn = small.tile([P, GT], F32, tag="kwmn")
            selu = small.tile([P, GT], U8, tag="kwselu")
            trash = scr.tile([P, n], BF16, tag=f"kwA{n}", name="trash")
            trashD = scr.tile([P, n], U8, tag=f"kwB{n}", name="trashD")

            nc.gpsimd.tensor_scalar(kp1[:], kk[:], 1.0, None, op0=OP.add)
            nc.gpsimd.memset(chA[:], 0.0)
            for t in range(GT):
                nc.vector.reduce_max(hiA[:, t:t + 1], zg[:, t, :], axis=AX.X)
                nc.vector.tensor_reduce(out=mn[:, t:t + 1], in_=zg[:, t, :],
                                        op=OP.min, axis=AX.X)
            nc.gpsimd.tensor_scalar(loA[:], mn[:], 1.0, None, op0=OP.subtract)
            yield

            lo, hi, ch = loA, hiA, chA
            lon, hin, chn = loB, hiB, chB
            for it in range(I):
                nc.gpsimd.tensor_tensor(msum[:], lo[:], hi[:], op=OP.add)
                nc.gpsimd.tensor_scalar(mid[:], msum[:], 0.5, None,
                                        op0=OP.mult)
                nc.gpsimd.tensor_scalar(nbias[:], mid[:], -SCALE, None,
                                        op0=OP.mult)
                for t in range(GT):
                    nc.scalar.activation(
                        trash[:], zg[:, t, :], AF.Sigmoid,
                        bias=nbias[:, t:t + 1], scale=SCALE,
                        accum_out=cnt[:, t:t + 1])
                nc.vector.tensor_tensor(selu[:], cnt[:], kp1[:], op=OP.is_ge)
                nc.vector.select(lon[:], selu[:], mid[:], lo[:])
                nc.vector.select(hin[:], selu[:], hi[:], mid[:])
                nc.vector.select(chn[:], selu[:], ch[:], cnt[:])
                lo, lon = lon, lo
                hi, hin = hin, hi
                ch, chn = chn, ch
                yield

            chii = small.tile([P, GT], I32, tag="kwchii")
            nc.vector.tensor_scalar(chn[:], ch[:], 0.25, None, op0=OP.subtract)
            nc.vector.tensor_copy(chii[:], chn[:])
            nc.vector.tensor_copy(ch[:], chii[:])
            rm1 = small.tile([P, GT], F32, tag="kwrm1")
            nc.vector.tensor_tensor(rm1[:], kk[:], ch[:], op=OP.subtract)
            yield

            for t in range(GT):
                m1 = scr.tile([P, n], F32, tag=f"kwA{n}", name="m1")
                gu8 = scr.tile([P, n], U8, tag=f"kwgu{n}", name="gu8")
                msk = scr.tile([P, n], F32, tag=f"kwmsk{n}", name="msk")
                nc.gpsimd.tensor_scalar(m1[:], zg[:, t, :], lo[:, t:t + 1],
                                        None, op0=OP.max)
                nc.vector.tensor_scalar(gu8[:], zg[:, t, :], hi[:, t:t + 1],
                                        None, op0=OP.is_gt)
                nc.vector.select(msk[:], gu8[:], negbig[:].to_broadcast([P, n]),
                                 m1[:])
                m8 = small.tile([P, 8], F32, tag="kwm8")
                nc.vector.max(out=m8[:], in_=msk[:])
                eq = small.tile([P, 8], F32, tag="kweq")
                nc.vector.tensor_scalar(eq[:], iota8[:], rm1[:, t:t + 1],
                                        None, op0=OP.is_equal)
                pr = small.tile([P, 8], F32, tag="kwpr")
                nc.vector.tensor_tensor(pr[:], eq[:], m8[:], op=OP.mult)
                u = small.tile([P, 1], F32, tag="kwu")
                nc.vector.reduce_sum(u[:], pr[:], axis=AX.X)
                yield
                geu = scr.tile([P, n], U8, tag=f"kwgu{n}", name="geu")
                nc.vector.tensor_scalar(geu[:], zg[:, t, :], u[:], None,
                                        op0=OP.is_gt)
                zth = scr.tile([P, n], F32, tag=f"kwA{n}", name="zth")
                nc.gpsimd.tensor_scalar(zth[:], zg[:, t, :], THIRD, None,
                                        op0=OP.mult)
                nc.vector.select(xg[:, t, :], geu[:], zg[:, t, :], zth[:])
                yield

        # transpose [P, GT, n] -> xT [P, n//P, BG]
        def transpose_x(xg, xT, n):
            nch = n // P
            for t in range(GT):
                for c0 in range(0, nch, 4):
                    cw = min(4, nch - c0)
                    ps = pst.tile([P, 4 * P], F32, tag="pst")
                    for c in range(c0, c0 + cw):
                        nc.tensor.transpose(
                            ps[:, (c - c0) * P:(c - c0 + 1) * P],
                            xg[:, t, c * P:(c + 1) * P], ident[:])
                    dst = xT[:, c0:c0 + cw, t * P:(t + 1) * P]
                    src = ps[:, 0:cw * P].rearrange("p (c q) -> p c q", q=P)
                    nc.vector.tensor_copy(dst, src)
                    yield

        # ---------------- phase B1: kwta1, x1T, l2 ---------------------------
        def phase_b1(g, st):
            x1 = shared.tile([P, GT, HID], F32, tag="big16", name="x1")
            yield from kwta(st["z1"], x1, st["kk0"], HID)
            x1T = shared.tile([P, HID // P, BG], F32, tag="hx1", name="x1T")
            yield from transpose_x(x1, x1T, HID)
            z2 = shared.tile([P, GT, HID2], F32, tag="z2")
            st["z2"] = z2
            ps = psb.tile([P, GT, 512], F32, tag="psb")
            for k in range(HID // P):
                wb = wslab_b("l2", k, 0, HID2)
                for t in range(GT):
                    nc.tensor.matmul(
                        ps[:, t, :], x1T[:, k, t * P:(t + 1) * P],
                        wb[:, 0, :], start=(k == 0), stop=(k == HID // P - 1))
                yield
            for t in range(GT):
                nc.vector.scalar_tensor_tensor(
                    z2[:, t, :], ps[:, t, :], 1.0, breps["l2"][:],
                    op0=OP.mult, op1=OP.add)
            yield

        # ---------------- phase B2: kwta2, x2T, l3 ---------------------------
        def phase_b2(g, st):
            x2 = big.tile([P, GT, HID2], F32, tag="x2")
            yield from kwta(st["z2"], x2, st["kk1"], HID2)
            x2T = shared.tile([P, HID2 // P, BG], F32, tag="hx2", name="x2T")
            yield from transpose_x(x2, x2T, HID2)
            ps3 = psa.tile([P, BG], F32, tag="psa")
            wa = wslab_a("l3", 0, HID2 // P, 0, P)
            for k in range(HID2 // P):
                nc.tensor.matmul(ps3[:], wa[:, k, :], x2T[:, k, :],
                                 start=(k == 0), stop=(k == HID2 // P - 1))
            z3T = big.tile([P, BG], F32, tag="zot", name="z3T")
            nc.vector.scalar_tensor_tensor(
                z3T[:], ps3[:], 1.0, bcols["l3"][:].to_broadcast([P, BG]),
                op0=OP.mult, op1=OP.add)
            yield
            z3 = shared.tile([P, GT, HEADS], F32, tag="z3")
            st["z3"] = z3
            for t in range(GT):
                pt = pst.tile([P, 4 * P], F32, tag="pst")
                nc.tensor.transpose(pt[:, 0:P], z3T[:, t * P:(t + 1) * P],
                                    ident[:])
                nc.vector.tensor_copy(z3[:, t, :], pt[:, 0:P])
            yield

        # ---------------- phase B3: kwta3, x3T, l4, out ----------------------
        def phase_b3(g, st):
            col0 = g * BG
            x3 = big.tile([P, GT, HEADS], F32, tag="x3")
            yield from kwta(st["z3"], x3, st["kk2"], HEADS)
            x3T = shared.tile([P, 1, BG], F32, tag="hx3", name="x3T")
            yield from transpose_x(x3, x3T, HEADS)
            ps4 = psa.tile([P, BG], F32, tag="psa")
            wa = wslab_a("l4", 0, 1, 0, P)
            nc.tensor.matmul(ps4[:], wa[:, 0, :], x3T[:, 0, :],
                             start=True, stop=True)
            og = big.tile([P, BG], F32, tag="zot", name="og")
            nc.vector.scalar_tensor_tensor(
                og[:], ps4[:], 1.0, bcols["l4"][:].to_broadcast([P, BG]),
                op0=OP.mult, op1=OP.add)
            nc.sync.dma_start(outT[:, col0:col0 + BG], og[:])
            yield

        # ---------------- weave ------------------------------------------
        sts = [dict() for _ in range(NG)]

        def weave(gens):
            active = list(gens)
            while active:
                keep = []
                for it in active:
                    try:
                        next(it)
                        keep.append(it)
                    except StopIteration:
                        pass
                active = keep

        def phase_a(g, st):
            yield from phase_a1(g, st)
            yield from phase_a2(g, st)

        def seq(*gens):
            for gi in gens:
                yield from gi

        slots = [
            [phase_a(0, sts[0])],
            [phase_a(1, sts[1])],
            [phase_a(2, sts[2]), phase_b1(0, sts[0])],
            [phase_a(3, sts[3]), phase_b2(0, sts[0]), phase_b1(1, sts[1])],
            [phase_b3(0, sts[0]), phase_b2(1, sts[1]),
             seq(phase_b1(2, sts[2]), phase_b1(3, sts[3]))],
            [phase_b3(1, sts[1]),
             seq(phase_b2(2, sts[2]), phase_b2(3, sts[3]))],
            [seq(phase_b3(2, sts[2]), phase_b3(3, sts[3]))],
        ]
        for s in slots:
            weave(s)


# ----------------------------------------------------------------------------
# host wrapper
# ----------------------------------------------------------------------------

_CACHE = {}


def _get_program():
    if "nc" not in _CACHE:
        _CACHE["nc"] = build_program()
    return _CACHE["nc"]


def _fingerprint(arrs):
    out = []
    for a in arrs:
        out.append((id(a), a.shape, a.dtype.str,
                    float(a.flat[0]), float(a.flat[-1])))
    return tuple(out)


def _prep_weights(ws):
    """ws: dict name -> (w, b). Returns the replicated input map (cached)."""
    arrs = [a for pair in ws.values() for a in pair]
    key = _fingerprint(arrs)
    hit = _CACHE.get("wkey")
    if hit == key:
        return _CACHE["wmap"]
    m = {}
    for name, (w, b) in ws.items():
        w = np.asarray(w, dtype=np.float32)
        b = np.asarray(b, dtype=np.float32)
        if name in IN_LAYERS:
            m[f"{name}_wT"] = np.ascontiguousarray(w[:, :1024].T)
            m[f"{name}_tail"] = np.ascontiguousarray(
                np.vstack([w[:, 1024:1028].T, b[None, :]]))
        else:
            m[f"{name}_wT"] = np.ascontiguousarray(w.T)
            if name in ("l3", "l4"):
                m[f"{name}_bcol"] = np.ascontiguousarray(
                    np.broadcast_to(b[:, None], (P, 1)))
            else:
                m[f"{name}_brep"] = np.ascontiguousarray(
                    np.broadcast_to(b[None, :], (P, b.shape[0])))
    _CACHE["wkey"] = key
    _CACHE["wmap"] = m
    return m


def kernel(**inputs):
    _trace = bool(inputs.pop("_trace", False))
    nc = _get_program()
    state = np.asarray(inputs["state"], dtype=np.float32)
    task = np.asarray(inputs["task_indicator"], dtype=np.float32)
    ws = {n: (inputs[f"{n}_w"], inputs[f"{n}_b"])
          for n in list(IN_LAYERS) + list(HID_LAYERS)}
    common = _prep_weights(ws)
    in_maps = []
    for c in range(NCORES):
        m = dict(common)
        m["state"] = state[c * BC:(c + 1) * BC]
        m["task"] = task[c * BC:(c + 1) * BC]
        in_maps.append(m)
    res = run_bass_kernel_spmd(nc, in_maps, core_ids=list(range(NCORES)),
                               trace=_trace)
    kernel.last_exec_time_ns = res.exec_time_ns
    out = np.concatenate([r["outT"].T for r in res.results], axis=0)
    return np.ascontiguousarray(out, dtype=np.float32)


kernel.last_exec_time_ns = None


# revision 27
# speedup vs baseline: 5.6507x; 2.0183x over previous
"""Trainium2 Bass kernel for nn_NeuralNetwork_S (kwta / topk_masking) — v2.

Strategy vs v1:
- Native fp32 matmuls (probe: max rel err 1.8e-7, same as v1's 3-term f32r
  split) -> no hi/lo splits anywhere: half the shipped bytes, no host-side
  rne12, no DVE subtract passes.
- Host ships raw per-core row slices of state/task (zero-copy views) and
  cached fp32 w^T; ci transpose happens on device (PE transpose + Pool copy).
- Biases of the 4 IN-facing layers fold into an augmented K=5 tail matmul
  (task^T rows + ones row) x (w_tail rows + bias row) — free on PE since
  matmul cost is N-driven.
- Software-pipelined emission: per group g, phase A (l1 + cx chains) and
  phases B1/B2/B3 (kwta1+l2 / kwta2+l3 / kwta3+l4) are emitted as woven
  generators so group g's kwta bisections (ACT/Pool/DVE) hide under group
  g+1's matmul stream (PE).
- kwta bisection: counts on ACT (sigmoid step w/ 2^100 scale, exact),
  interval ping-pong small ops split DVE/Pool to stay within the 4-deep
  wait stations.
"""

import sys

_TRN = "/opt/trn_rl_repo"
if _TRN not in sys.path:
    sys.path.insert(0, _TRN)

import numpy as np
import concourse.bass as bass
import concourse.mybir as mybir
import concourse.tile as tile
from concourse import bacc
from concourse.bass_utils import run_bass_kernel_spmd
from concourse.masks import make_identity

P = 128
B = 16384
NCORES = 8
BC = B // NCORES          # 2048 rows per core
BG = 512                  # rows per group
NG = BC // BG             # 4 groups
GT = BG // P              # 4 row-tiles per group
IN = 1028
KIN = 8                   # full 128-row k-chunks of the 1024 state features
HID = 1024
HID2 = 512
HEADS = 128

F32 = mybir.dt.float32
U8 = mybir.dt.uint8
I32 = mybir.dt.int32
U32 = mybir.dt.uint32
BF16 = mybir.dt.bfloat16
AF = mybir.ActivationFunctionType
OP = mybir.AluOpType
AX = mybir.AxisListType

SCALE = float(2.0 ** 100)
ITERS = {1024: 12, 512: 12, 128: 10}
THIRD = 1.0 / 3.0

# layer tables ---------------------------------------------------------------
# IN-layers (read ci): (name, out, form); form 'a' = out-on-partitions,
# 'b' = rows-on-partitions
IN_LAYERS = {"cx11": HID, "cx21": HID2, "cx31": HEADS, "l1": HID}
# hidden layers: name -> (k_in, out)
HID_LAYERS = {"cx12": (HID, HID), "cx22": (HID2, HID2), "cx32": (HEADS, HEADS),
              "l2": (HID, HID2), "l3": (HID2, HEADS), "l4": (HEADS, HEADS)}
W_DIMS = {"l1": (1024, HID), "cx11": (1024, HID), "cx12": (HID, HID),
          "cx21": (1024, HID2), "cx22": (HID2, HID2), "cx31": (1024, HEADS),
          "cx32": (HEADS, HEADS), "l2": (HID, HID2), "l3": (HID2, HEADS),
          "l4": (HEADS, HEADS)}
# (name, half_id, col0, width) in gather order (= first-use order)
W_GATHERS = [("l1", 0, 0, 512), ("l1", 1, 512, 512),
             ("cx11", 0, 0, 512), ("cx11", 1, 512, 512),
             ("cx12", 0, 0, 512), ("cx12", 1, 512, 512),
             ("cx21", 0, 0, 256), ("cx21", 1, 256, 256),
             ("cx22", 0, 0, 512), ("cx31", 0, 0, 128),
             ("cx32", 0, 0, 128), ("l2", 0, 0, 512),
             ("l3", 0, 0, 128), ("l4", 0, 0, 128)]


def build_program():
    nc = bacc.Bacc("TRN2", target_bir_lowering=False, debug=False)
    d = {}

    def din(name, shape, dt=F32):
        d[name] = nc.dram_tensor(name, list(shape), dt, kind="ExternalInput")
        return d[name]

    din("state", [BC, 1024])
    din("task", [BC, 4])
    for name, o in IN_LAYERS.items():
        din(f"{name}_tail", [5, o])
    for name, half, o0, ow in W_GATHERS:
        din(f"{name}_ws{half}", [W_DIMS[name][0] // NCORES, ow])
    for name in ("cx12", "cx22", "cx32", "l2"):
        din(f"{name}_brep", [P, HID_LAYERS[name][1]])
    for name in ("l3", "l4"):
        din(f"{name}_bcol", [P, 1])

    outT = nc.dram_tensor("outT", [P, BC], F32, kind="ExternalOutput")

    with tile.TileContext(nc) as tc:
        _emit(tc, nc, d, outT)
    nc.compile()
    return nc


def _emit(tc, nc, d, outT):
    import contextlib

    ctx = contextlib.ExitStack()
    with ctx:
        big = ctx.enter_context(tc.tile_pool(name="big", bufs=1))
        dbuf = ctx.enter_context(tc.tile_pool(name="dbuf", bufs=1))
        shared = ctx.enter_context(tc.tile_pool(name="shared", bufs=2))
        wts = ctx.enter_context(tc.tile_pool(name="wts", bufs=2))
        cons = ctx.enter_context(tc.tile_pool(name="cons", bufs=1))
        small = ctx.enter_context(tc.tile_pool(name="small", bufs=4))
        scr = ctx.enter_context(tc.tile_pool(name="scr", bufs=1))
        dram = ctx.enter_context(tc.tile_pool(name="dram", bufs=1,
                                               space="DRAM"))
        psb = ctx.enter_context(tc.tile_pool(name="psb", bufs=1, space="PSUM"))
        psa = ctx.enter_context(tc.tile_pool(name="psa", bufs=2, space="PSUM"))
        pst = ctx.enter_context(tc.tile_pool(name="pst", bufs=2, space="PSUM"))

        # constants ----------------------------------------------------------
        ident = cons.tile([P, P], F32, tag="ident")
        make_identity(nc, ident[:])
        negbig = cons.tile([P, 1], F32, tag="negbig")
        nc.vector.memset(negbig[:], -1.0e30)
        iota8 = cons.tile([P, 8], F32, tag="iota8")
        iota8u = small.tile([P, 8], U32, tag="iota8u")
        nc.gpsimd.iota(iota8u[:], pattern=[[1, 8]], base=0, channel_multiplier=0)
        nc.vector.tensor_copy(iota8[:], iota8u[:])
        zbias = cons.tile([P, 1], F32, tag="zbias")
        nc.vector.memset(zbias[:], 0.0)

        # resident weights: tails + breps + bcols -----------------------------
        tails = {}
        for name, o in IN_LAYERS.items():
            t = cons.tile([5, o], F32, tag=f"tail_{name}")
            nc.sync.dma_start(t[:], d[f"{name}_tail"][:])
            tails[name] = t
        breps = {}
        for name in ("cx12", "cx22", "cx32", "l2"):
            t = cons.tile([P, HID_LAYERS[name][1]], F32, tag=f"brep_{name}")
            nc.sync.dma_start(t[:], d[f"{name}_brep"][:])
            breps[name] = t
        bcols = {}
        for name in ("l3", "l4"):
            t = cons.tile([P, 1], F32, tag=f"bcol_{name}")
            nc.sync.dma_start(t[:], d[f"{name}_bcol"][:])
            bcols[name] = t

        state_r = d["state"].rearrange("(n p) f -> p n f", p=P)   # [P,16,1024]
        task_r = d["task"].rearrange("(n p) f -> p n f", p=P)     # [P,16,4]

        # ---- weight all-gather: each core ships 1/8 of each wT.
        # Big layers gather in column halves, ordered by first use, so the
        # gathers pipeline against group-0 compute.
        gathered = {}
        for name, half, o0, ow in W_GATHERS:
            k = W_DIMS[name][0]
            bin_ = dram.tile([k // NCORES, ow], F32, tag=f"gin_{name}{half}")
            bout = dram.tile([k, ow], F32, tag=f"gout_{name}{half}")
            nc.sync.dma_start(bin_[:], d[f"{name}_ws{half}"][:])
            nc.gpsimd.collective_compute(
                "AllGather", mybir.AluOpType.bypass,
                replica_groups=[list(range(NCORES))],
                ins=[bin_.opt()], outs=[bout.opt()])
            gathered[(name, half)] = bout

        def gsrc(name, c0, cw):
            """Gathered wT cols [c0, c0+cw) -> (tile, local col offset)."""
            for nm, half, o0, ow in W_GATHERS:
                if nm == name and o0 <= c0 and c0 + cw <= o0 + ow:
                    return gathered[(name, half)], c0 - o0
            raise KeyError((name, c0, cw))

        def wslab_b(name, k, n0, nw):
            """(b)-form moving slab [P, 1, nw] from wT rows [k*128, +128)."""
            t = wts.tile([P, 1, nw], F32, tag="wb")
            g, off = gsrc(name, n0, nw)
            src = g[:].rearrange("(c p) o -> p c o", p=P)
            nc.sync.dma_start(t[:], src[:, k:k + 1, off:off + nw])
            return t

        def wslab_a(name, k0, kc, m0, mw):
            """(a)-form stationary slab [P, kc<=4, mw] (k-chunks k0..k0+kc)."""
            t = wts.tile([P, kc, mw], F32, tag="wa")
            g, off = gsrc(name, m0, mw)
            src = g[:].rearrange("(c p) o -> p c o", p=P)
            nc.sync.dma_start(t[:], src[:, k0:k0 + kc, off:off + mw])
            return t

        # ---------------- phase A1: ci transpose, l1, cx1 chain --------------
        def phase_a1(g, st):
            col0 = g * BG
            ciT = shared.tile([P, KIN, BG], F32, tag="big16", name="ciT")
            st["ciT"] = ciT
            taskT = big.tile([5, BG], F32, tag="taskT")
            st["taskT"] = taskT
            tTASK = small.tile([P, GT, 5], F32, tag="tTASK")
            nc.sync.dma_start(tTASK[:, :, 0:4], task_r[:, g * GT:(g + 1) * GT, :])
            nc.gpsimd.memset(tTASK[:, :, 4:5], 1.0)
            yield
            # transpose ci into [feature-part, row] layout
            for t in range(GT):
                sROW = dbuf.tile([P, 1024], F32, tag="sROW")
                nc.sync.dma_start(sROW[:], state_r[:, g * GT + t, :])
                for c0 in (0, 4):
                    ps = pst.tile([P, 4 * P], F32, tag="pst")
                    for c in range(c0, c0 + 4):
                        nc.tensor.transpose(
                            ps[:, (c - c0) * P:(c - c0 + 1) * P],
                            sROW[:, c * P:(c + 1) * P], ident[:])
                    dst = ciT[:, c0:c0 + 4, t * P:(t + 1) * P]
                    src = ps[:].rearrange("p (c q) -> p c q", q=P)
                    nc.vector.tensor_copy(dst, src)
                    yield
                pt = pst.tile([P, 4 * P], F32, tag="pst")
                nc.tensor.transpose(pt[0:5, 0:P], tTASK[:, t, :], ident[:])
                nc.vector.tensor_copy(taskT[0:5, t * P:(t + 1) * P],
                                      pt[0:5, 0:P])
                yield

            # ---- l1 (b): z1 [P, GT, 1024]
            z1 = shared.tile([P, GT, HID], F32, tag="z1", name="z1")
            st["z1"] = z1
            for n0 in range(0, HID, 512):
                ps = psb.tile([P, GT, 512], F32, tag="psb")
                for k in range(KIN):
                    wb = wslab_b("l1", k, n0, 512)
                    for t in range(GT):
                        nc.tensor.matmul(
                            ps[:, t, :], ciT[:, k, t * P:(t + 1) * P],
                            wb[:, 0, :], start=(k == 0), stop=False)
                    yield
                for t in range(GT):
                    nc.tensor.matmul(
                        ps[:, t, :], taskT[0:5, t * P:(t + 1) * P],
                        tails["l1"][0:5, n0:n0 + 512], start=False, stop=True)
                yield
                for t in range(GT):
                    nc.vector.tensor_copy(z1[:, t, n0:n0 + 512], ps[:, t, :])
                yield

            # ---- cx1 chain -> kk0
            yield from cx_chain(g, st, 0)

        # ---------------- phase A2: cx2/cx3 chains ---------------------------
        def phase_a2(g, st):
            yield from cx_chain(g, st, 1)
            yield from cx_chain(g, st, 2)

        CX_DEFS = [("cx11", "cx12", HID, 8), ("cx21", "cx22", HID2, 4),
                   ("cx31", "cx32", HEADS, 1)]

        def cx_chain(g, st, cn):
            ciT = st["ciT"]
            taskT = st["taskT"]
            if True:
                pre, post, hidn, mch = CX_DEFS[cn]
                kc_pre = KIN
                httag = {0: "hx1", 1: "hx2", 2: "hx3"}[cn]
                hT = shared.tile([P, mch, BG], F32, tag=httag, name=f"hT{cn}")
                for m in range(mch):
                    ps = psa.tile([P, BG], F32, tag="psa")
                    for k0 in range(0, kc_pre, 4):
                        wa = wslab_a(pre, k0, 4, m * P, P)
                        for k in range(k0, k0 + 4):
                            nc.tensor.matmul(ps[:], wa[:, k - k0, :],
                                             ciT[:, k, :],
                                             start=(k == 0), stop=False)
                    nc.tensor.matmul(ps[:], tails[pre][0:5, m * P:(m + 1) * P],
                                     taskT[0:5, :], start=False, stop=True)
                    nc.scalar.activation(hT[:, m, :], ps[:], AF.Tanh,
                                         bias=zbias[:], scale=1.0)
                    yield

                # second layer (b) + incremental argmax
                kk = small.tile([P, GT], F32, tag=f"kk{cn}", name="kk")
                st[f"kk{cn}"] = kk
                kin2, out2 = HID_LAYERS[post]
                bestm = small.tile([P, GT], F32, tag="bestm")
                kkA = small.tile([P, GT], F32, tag="kkA")
                n0s = list(range(0, out2, 512))
                for ci_, n0 in enumerate(n0s):
                    nw = min(512, out2)
                    ps = psb.tile([P, GT, 512], F32, tag="psb")
                    for k in range(mch):
                        wb = wslab_b(post, k, n0, nw)
                        for t in range(GT):
                            nc.tensor.matmul(
                                ps[:, t, 0:nw], hT[:, k, t * P:(t + 1) * P],
                                wb[:, 0, :], start=(k == 0), stop=(k == mch - 1))
                        yield
                    m8 = small.tile([P, 8], F32, tag="am8")
                    idx = small.tile([P, 8], U32, tag="aidx")
                    idxf = small.tile([P, 8], F32, tag="aidxf")
                    for t in range(GT):
                        zcx = big.tile([P, 512], F32, tag="zcx", name="zcx")
                        nc.vector.scalar_tensor_tensor(
                            zcx[:, 0:nw], ps[:, t, 0:nw], 1.0,
                            breps[post][:, n0:n0 + nw], op0=OP.mult, op1=OP.add)
                        nc.vector.max(out=m8[:], in_=zcx[:, 0:nw])
                        nc.vector.max_index(idx[:], m8[:], zcx[:, 0:nw])
                        nc.vector.tensor_copy(idxf[:, 0:1], idx[:, 0:1])
                        if ci_ == 0 and len(n0s) == 1:
                            nc.vector.tensor_copy(kk[:, t:t + 1], idxf[:, 0:1])
                        elif ci_ == 0:
                            nc.vector.tensor_copy(kkA[:, t:t + 1], idxf[:, 0:1])
                            nc.vector.tensor_copy(bestm[:, t:t + 1],
                                                  m8[:, 0:1])
                        else:
                            gtu = small.tile([P, 1], U8, tag="agt")
                            nc.vector.tensor_tensor(
                                gtu[:], m8[:, 0:1], bestm[:, t:t + 1],
                                op=OP.is_gt)
                            i2 = small.tile([P, 1], F32, tag="ai2")
                            nc.vector.tensor_scalar(
                                i2[:], idxf[:, 0:1], float(n0), None,
                                op0=OP.add)
                            nc.vector.select(kk[:, t:t + 1], gtu[:], i2[:],
                                             kkA[:, t:t + 1])
                        yield

        # ---------------- kwta bisection ------------------------------------
        def kwta(zg, xg, kk, n):
            I = ITERS[n]
            loA = small.tile([P, GT], F32, tag="kwloA")
            loB = small.tile([P, GT], F32, tag="kwloB")
            hiA = small.tile([P, GT], F32, tag="kwhiA")
            hiB = small.tile([P, GT], F32, tag="kwhiB")
            chA = small.tile([P, GT], F32, tag="kwchA")
            chB = small.tile([P, GT], F32, tag="kwchB")
            cnt = small.tile([P, GT], F32, tag="kwcnt")
            kp1 = small.tile([P, GT], F32, tag="kwkp1")
            msum = small.tile([P, GT], F32, tag="kwmsum")
            mid = small.tile([P, GT], F32, tag="kwmid")
            nbias = small.tile([P, GT], F32, tag="kwnb")
            mn = small.tile([P, GT], F32, tag="kwmn")
            selu = small.tile([P, GT], U8, tag="kwselu")
            trash = scr.tile([P, n], BF16, tag=f"kwA{n}", name="trash")
            trashD = scr.tile([P, n], U8, tag=f"kwB{n}", name="trashD")

            nc.gpsimd.tensor_scalar(kp1[:], kk[:], 1.0, None, op0=OP.add)
            nc.gpsimd.memset(chA[:], 0.0)
            for t in range(GT):
                nc.vector.reduce_max(hiA[:, t:t + 1], zg[:, t, :], axis=AX.X)
                nc.vector.tensor_reduce(out=mn[:, t:t + 1], in_=zg[:, t, :],
                                        op=OP.min, axis=AX.X)
            nc.gpsimd.tensor_scalar(loA[:], mn[:], 1.0, None, op0=OP.subtract)
            yield

            lo, hi, ch = loA, hiA, chA
            lon, hin, chn = loB, hiB, chB
            for it in range(I):
                nc.gpsimd.tensor_tensor(msum[:], lo[:], hi[:], op=OP.add)
                nc.gpsimd.tensor_scalar(mid[:], msum[:], 0.5, None,
                                        op0=OP.mult)
                nc.gpsimd.tensor_scalar(nbias[:], mid[:], -SCALE, None,
                                        op0=OP.mult)
                for t in range(GT):
                    nc.scalar.activation(
                        trash[:], zg[:, t, :], AF.Sigmoid,
                        bias=nbias[:, t:t + 1], scale=SCALE,
                        accum_out=cnt[:, t:t + 1])
                nc.vector.tensor_tensor(selu[:], cnt[:], kp1[:], op=OP.is_ge)
                nc.vector.select(lon[:], selu[:], mid[:], lo[:])
                nc.vector.select(hin[:], selu[:], hi[:], mid[:])
                nc.vector.select(chn[:], selu[:], ch[:], cnt[:])
                lo, lon = lon, lo
                hi, hin = hin, hi
                ch, chn = chn, ch
                yield

            chii = small.tile([P, GT], I32, tag="kwchii")
            nc.vector.tensor_scalar(chn[:], ch[:], 0.25, None, op0=OP.subtract)
            nc.vector.tensor_copy(chii[:], chn[:])
            nc.vector.tensor_copy(ch[:], chii[:])
            rm1 = small.tile([P, GT], F32, tag="kwrm1")
            nc.vector.tensor_tensor(rm1[:], kk[:], ch[:], op=OP.subtract)
            yield

            for t in range(GT):
                m1 = scr.tile([P, n], F32, tag=f"kwA{n}", name="m1")
                gu8 = scr.tile([P, n], U8, tag=f"kwgu{n}", name="gu8")
                msk = scr.tile([P, n], F32, tag=f"kwmsk{n}", name="msk")
                nc.gpsimd.tensor_scalar(m1[:], zg[:, t, :], lo[:, t:t + 1],
                                        None, op0=OP.max)
                nc.vector.tensor_scalar(gu8[:], zg[:, t, :], hi[:, t:t + 1],
                                        None, op0=OP.is_gt)
                nc.vector.select(msk[:], gu8[:], negbig[:].to_broadcast([P, n]),
                                 m1[:])
                m8 = small.tile([P, 8], F32, tag="kwm8")
                nc.vector.max(out=m8[:], in_=msk[:])
                eq = small.tile([P, 8], F32, tag="kweq")
                nc.vector.tensor_scalar(eq[:], iota8[:], rm1[:, t:t + 1],
                                        None, op0=OP.is_equal)
                pr = small.tile([P, 8], F32, tag="kwpr")
                nc.vector.tensor_tensor(pr[:], eq[:], m8[:], op=OP.mult)
                u = small.tile([P, 1], F32, tag="kwu")
                nc.vector.reduce_sum(u[:], pr[:], axis=AX.X)
                yield
                geu = scr.tile([P, n], U8, tag=f"kwgu{n}", name="geu")
                nc.vector.tensor_scalar(geu[:], zg[:, t, :], u[:], None,
                                        op0=OP.is_gt)
                zth = scr.tile([P, n], F32, tag=f"kwA{n}", name="zth")
                nc.gpsimd.tensor_scalar(zth[:], zg[:, t, :], THIRD, None,
                                        op0=OP.mult)
                nc.vector.select(xg[:, t, :], geu[:], zg[:, t, :], zth[:])
                yield

        # transpose [P, GT, n] -> xT [P, n//P, BG]
        def transpose_x(xg, xT, n):
            nch = n // P
            for t in range(GT):
                for c0 in range(0, nch, 4):
                    cw = min(4, nch - c0)
                    ps = pst.tile([P, 4 * P], F32, tag="pst")
                    for c in range(c0, c0 + cw):
                        nc.tensor.transpose(
                            ps[:, (c - c0) * P:(c - c0 + 1) * P],
                            xg[:, t, c * P:(c + 1) * P], ident[:])
                    dst = xT[:, c0:c0 + cw, t * P:(t + 1) * P]
                    src = ps[:, 0:cw * P].rearrange("p (c q) -> p c q", q=P)
                    nc.vector.tensor_copy(dst, src)
                    yield

        # ---------------- phase B1: kwta1, x1T, l2 ---------------------------
        def phase_b1(g, st):
            x1 = shared.tile([P, GT, HID], F32, tag="big16", name="x1")
            yield from kwta(st["z1"], x1, st["kk0"], HID)
            x1T = shared.tile([P, HID // P, BG], F32, tag="hx1", name="x1T")
            yield from transpose_x(x1, x1T, HID)
            z2 = shared.tile([P, GT, HID2], F32, tag="z2")
            st["z2"] = z2
            ps = psb.tile([P, GT, 512], F32, tag="psb")
            for k in range(HID // P):
                wb = wslab_b("l2", k, 0, HID2)
                for t in range(GT):
                    nc.tensor.matmul(
                        ps[:, t, :], x1T[:, k, t * P:(t + 1) * P],
                        wb[:, 0, :], start=(k == 0), stop=(k == HID // P - 1))
                yield
            for t in range(GT):
                nc.vector.scalar_tensor_tensor(
                    z2[:, t, :], ps[:, t, :], 1.0, breps["l2"][:],
                    op0=OP.mult, op1=OP.add)
            yield

        # ---------------- phase B2: kwta2, x2T, l3 ---------------------------
        def phase_b2(g, st):
            x2 = big.tile([P, GT, HID2], F32, tag="x2")
            yield from kwta(st["z2"], x2, st["kk1"], HID2)
            x2T = shared.tile([P, HID2 // P, BG], F32, tag="hx2", name="x2T")
            yield from transpose_x(x2, x2T, HID2)
            ps3 = psa.tile([P, BG], F32, tag="psa")
            wa = wslab_a("l3", 0, HID2 // P, 0, P)
            for k in range(HID2 // P):
                nc.tensor.matmul(ps3[:], wa[:, k, :], x2T[:, k, :],
                                 start=(k == 0), stop=(k == HID2 // P - 1))
            z3T = big.tile([P, BG], F32, tag="zot", name="z3T")
            nc.vector.scalar_tensor_tensor(
                z3T[:], ps3[:], 1.0, bcols["l3"][:].to_broadcast([P, BG]),
                op0=OP.mult, op1=OP.add)
            yield
            z3 = shared.tile([P, GT, HEADS], F32, tag="z3")
            st["z3"] = z3
            for t in range(GT):
                pt = pst.tile([P, 4 * P], F32, tag="pst")
                nc.tensor.transpose(pt[:, 0:P], z3T[:, t * P:(t + 1) * P],
                                    ident[:])
                nc.vector.tensor_copy(z3[:, t, :], pt[:, 0:P])
            yield

        # ---------------- phase B3: kwta3, x3T, l4, out ----------------------
        def phase_b3(g, st):
            col0 = g * BG
            x3 = big.tile([P, GT, HEADS], F32, tag="x3")
            yield from kwta(st["z3"], x3, st["kk2"], HEADS)
            x3T = shared.tile([P, 1, BG], F32, tag="hx3", name="x3T")
            yield from transpose_x(x3, x3T, HEADS)
            ps4 = psa.tile([P, BG], F32, tag="psa")
            wa = wslab_a("l4", 0, 1, 0, P)
            nc.tensor.matmul(ps4[:], wa[:, 0, :], x3T[:, 0, :],
                             start=True, stop=True)
            og = big.tile([P, BG], F32, tag="zot", name="og")
            nc.vector.scalar_tensor_tensor(
                og[:], ps4[:], 1.0, bcols["l4"][:].to_broadcast([P, BG]),
                op0=OP.mult, op1=OP.add)
            nc.sync.dma_start(outT[:, col0:col0 + BG], og[:])
            yield

        # ---------------- weave ------------------------------------------
        sts = [dict() for _ in range(NG)]

        def weave(gens):
            active = list(gens)
            while active:
                keep = []
                for it in active:
                    try:
                        next(it)
                        keep.append(it)
                    except StopIteration:
                        pass
                active = keep

        def phase_a(g, st):
            yield from phase_a1(g, st)
            yield from phase_a2(g, st)

        def seq(*gens):
            for gi in gens:
                yield from gi

        slots = [
            [phase_a(0, sts[0])],
            [phase_a(1, sts[1])],
            [phase_a(2, sts[2]), phase_b1(0, sts[0])],
            [phase_a(3, sts[3]), phase_b2(0, sts[0]), phase_b1(1, sts[1])],
            [phase_b3(0, sts[0]), phase_b2(1, sts[1]),
             seq(phase_b1(2, sts[2]), phase_b1(3, sts[3]))],
            [phase_b3(1, sts[1]),
             seq(phase_b2(2, sts[2]), phase_b2(3, sts[3]))],
            [seq(phase_b3(2, sts[2]), phase_b3(3, sts[3]))],
        ]
        for s in slots:
            weave(s)


# ----------------------------------------------------------------------------
# host wrapper
# ----------------------------------------------------------------------------

_CACHE = {}


def _get_program():
    if "nc" not in _CACHE:
        _CACHE["nc"] = build_program()
    return _CACHE["nc"]


def _fingerprint(arrs):
    out = []
    for a in arrs:
        out.append((id(a), a.shape, a.dtype.str,
                    float(a.flat[0]), float(a.flat[-1])))
    return tuple(out)


def _prep_weights(ws):
    """ws: dict name -> (w, b). Returns the replicated input map (cached)."""
    arrs = [a for pair in ws.values() for a in pair]
    key = _fingerprint(arrs)
    hit = _CACHE.get("wkey")
    if hit == key:
        return _CACHE["wmap"]
    m = {}
    shards = {}
    for name, (w, b) in ws.items():
        w = np.asarray(w, dtype=np.float32)
        b = np.asarray(b, dtype=np.float32)
        if name in IN_LAYERS:
            wT = np.ascontiguousarray(w[:, :1024].T)
            m[f"{name}_tail"] = np.ascontiguousarray(
                np.vstack([w[:, 1024:1028].T, b[None, :]]))
        else:
            wT = np.ascontiguousarray(w.T)
            if name in ("l3", "l4"):
                m[f"{name}_bcol"] = np.ascontiguousarray(
                    np.broadcast_to(b[:, None], (P, 1)))
            else:
                m[f"{name}_brep"] = np.ascontiguousarray(
                    np.broadcast_to(b[None, :], (P, b.shape[0])))
        kk = wT.shape[0] // NCORES
        for nm, half, o0, ow in W_GATHERS:
            if nm != name:
                continue
            wh = wT[:, o0:o0 + ow]
            if ow != wT.shape[1]:
                wh = np.ascontiguousarray(wh)
            shards[f"{name}_ws{half}"] = [wh[c * kk:(c + 1) * kk]
                                          for c in range(NCORES)]
    _CACHE["wkey"] = key
    _CACHE["wmap"] = (m, shards)
    return m, shards


def kernel(**inputs):
    _trace = bool(inputs.pop("_trace", False))
    nc = _get_program()
    state = np.asarray(inputs["state"], dtype=np.float32)
    task = np.asarray(inputs["task_indicator"], dtype=np.float32)
    ws = {n: (inputs[f"{n}_w"], inputs[f"{n}_b"])
          for n in list(IN_LAYERS) + list(HID_LAYERS)}
    common, shards = _prep_weights(ws)
    in_maps = []
    for c in range(NCORES):
        m = dict(common)
        m["state"] = state[c * BC:(c + 1) * BC]
        m["task"] = task[c * BC:(c + 1) * BC]
        for sk, sv in shards.items():
            m[sk] = sv[c]
        in_maps.append(m)
    res = run_bass_kernel_spmd(nc, in_maps, core_ids=list(range(NCORES)),
                               trace=_trace)
    kernel.last_exec_time_ns = res.exec_time_ns
    out = np.concatenate([r["outT"].T for r in res.results], axis=0)
    return np.ascontiguousarray(out, dtype=np.float32)


kernel.last_exec_time_ns = None


# revision 28
# speedup vs baseline: 5.7776x; 1.0225x over previous
"""Trainium2 Bass kernel for nn_NeuralNetwork_S (kwta / topk_masking) — v2.

Strategy vs v1:
- Native fp32 matmuls (probe: max rel err 1.8e-7, same as v1's 3-term f32r
  split) -> no hi/lo splits anywhere: half the shipped bytes, no host-side
  rne12, no DVE subtract passes.
- Host ships raw per-core row slices of state/task (zero-copy views) and
  cached fp32 w^T; ci transpose happens on device (PE transpose + Pool copy).
- Biases of the 4 IN-facing layers fold into an augmented K=5 tail matmul
  (task^T rows + ones row) x (w_tail rows + bias row) — free on PE since
  matmul cost is N-driven.
- Software-pipelined emission: per group g, phase A (l1 + cx chains) and
  phases B1/B2/B3 (kwta1+l2 / kwta2+l3 / kwta3+l4) are emitted as woven
  generators so group g's kwta bisections (ACT/Pool/DVE) hide under group
  g+1's matmul stream (PE).
- kwta bisection: counts on ACT (sigmoid step w/ 2^100 scale, exact),
  interval ping-pong small ops split DVE/Pool to stay within the 4-deep
  wait stations.
"""

import sys

_TRN = "/opt/trn_rl_repo"
if _TRN not in sys.path:
    sys.path.insert(0, _TRN)

import numpy as np
import concourse.bass as bass
import concourse.mybir as mybir
import concourse.tile as tile
from concourse import bacc
from concourse.bass_utils import run_bass_kernel_spmd
from concourse.masks import make_identity

P = 128
B = 16384
NCORES = 8
BC = B // NCORES          # 2048 rows per core
BG = 512                  # rows per group
NG = BC // BG             # 4 groups
GT = BG // P              # 4 row-tiles per group
IN = 1028
KIN = 8                   # full 128-row k-chunks of the 1024 state features
HID = 1024
HID2 = 512
HEADS = 128

F32 = mybir.dt.float32
U8 = mybir.dt.uint8
I32 = mybir.dt.int32
U32 = mybir.dt.uint32
BF16 = mybir.dt.bfloat16
AF = mybir.ActivationFunctionType
OP = mybir.AluOpType
AX = mybir.AxisListType

SCALE = float(2.0 ** 100)
ITERS = {1024: 12, 512: 12, 128: 10}
THIRD = 1.0 / 3.0

# layer tables ---------------------------------------------------------------
# IN-layers (read ci): (name, out, form); form 'a' = out-on-partitions,
# 'b' = rows-on-partitions
IN_LAYERS = {"cx11": HID, "cx21": HID2, "cx31": HEADS, "l1": HID}
# hidden layers: name -> (k_in, out)
HID_LAYERS = {"cx12": (HID, HID), "cx22": (HID2, HID2), "cx32": (HEADS, HEADS),
              "l2": (HID, HID2), "l3": (HID2, HEADS), "l4": (HEADS, HEADS)}
W_DIMS = {"l1": (1024, HID), "cx11": (1024, HID), "cx12": (HID, HID),
          "cx21": (1024, HID2), "cx22": (HID2, HID2), "cx31": (1024, HEADS),
          "cx32": (HEADS, HEADS), "l2": (HID, HID2), "l3": (HID2, HEADS),
          "l4": (HEADS, HEADS)}
W_ORDER = ["l1", "cx11", "cx12", "cx21", "cx22", "cx31", "cx32",
           "l2", "l3", "l4"]
W_OFF = {}
_off = 0
for _n in W_ORDER:
    W_OFF[_n] = _off
    _off += W_DIMS[_n][0] * W_DIMS[_n][1]
WTOT = _off          # 4,685,824 floats
WSH = WTOT // NCORES


def build_program():
    nc = bacc.Bacc("TRN2", target_bir_lowering=False, debug=False)
    d = {}

    def din(name, shape, dt=F32):
        d[name] = nc.dram_tensor(name, list(shape), dt, kind="ExternalInput")
        return d[name]

    din("state", [BC, 1024])
    din("task", [BC, 4])
    for name, o in IN_LAYERS.items():
        din(f"{name}_tail", [5, o])
    din("wflat_sh", [WSH])
    for name in ("cx12", "cx22", "cx32", "l2"):
        din(f"{name}_brep", [P, HID_LAYERS[name][1]])
    for name in ("l3", "l4"):
        din(f"{name}_bcol", [P, 1])

    outT = nc.dram_tensor("outT", [P, BC], F32, kind="ExternalOutput")

    with tile.TileContext(nc) as tc:
        _emit(tc, nc, d, outT)
    nc.compile()
    return nc


def _emit(tc, nc, d, outT):
    import contextlib

    ctx = contextlib.ExitStack()
    with ctx:
        big = ctx.enter_context(tc.tile_pool(name="big", bufs=1))
        dbuf = ctx.enter_context(tc.tile_pool(name="dbuf", bufs=1))
        shared = ctx.enter_context(tc.tile_pool(name="shared", bufs=2))
        wts = ctx.enter_context(tc.tile_pool(name="wts", bufs=2))
        cons = ctx.enter_context(tc.tile_pool(name="cons", bufs=1))
        small = ctx.enter_context(tc.tile_pool(name="small", bufs=4))
        scr = ctx.enter_context(tc.tile_pool(name="scr", bufs=1))
        dram = ctx.enter_context(tc.tile_pool(name="dram", bufs=1,
                                               space="DRAM"))
        psb = ctx.enter_context(tc.tile_pool(name="psb", bufs=1, space="PSUM"))
        psa = ctx.enter_context(tc.tile_pool(name="psa", bufs=2, space="PSUM"))
        pst = ctx.enter_context(tc.tile_pool(name="pst", bufs=2, space="PSUM"))

        # constants ----------------------------------------------------------
        ident = cons.tile([P, P], F32, tag="ident")
        make_identity(nc, ident[:])
        negbig = cons.tile([P, 1], F32, tag="negbig")
        nc.vector.memset(negbig[:], -1.0e30)
        iota8 = cons.tile([P, 8], F32, tag="iota8")
        iota8u = small.tile([P, 8], U32, tag="iota8u")
        nc.gpsimd.iota(iota8u[:], pattern=[[1, 8]], base=0, channel_multiplier=0)
        nc.vector.tensor_copy(iota8[:], iota8u[:])
        zbias = cons.tile([P, 1], F32, tag="zbias")
        nc.vector.memset(zbias[:], 0.0)

        # resident weights: tails + breps + bcols -----------------------------
        tails = {}
        for name, o in IN_LAYERS.items():
            t = cons.tile([5, o], F32, tag=f"tail_{name}")
            nc.sync.dma_start(t[:], d[f"{name}_tail"][:])
            tails[name] = t
        breps = {}
        for name in ("cx12", "cx22", "cx32", "l2"):
            t = cons.tile([P, HID_LAYERS[name][1]], F32, tag=f"brep_{name}")
            nc.sync.dma_start(t[:], d[f"{name}_brep"][:])
            breps[name] = t
        bcols = {}
        for name in ("l3", "l4"):
            t = cons.tile([P, 1], F32, tag=f"bcol_{name}")
            nc.sync.dma_start(t[:], d[f"{name}_bcol"][:])
            bcols[name] = t

        state_r = d["state"].rearrange("(n p) f -> p n f", p=P)   # [P,16,1024]
        task_r = d["task"].rearrange("(n p) f -> p n f", p=P)     # [P,16,4]

        # ---- weight all-gather: one flat 18.8MB gather (BW ramps with
        # size; 15us fixed overhead per collective favors a single one).
        gin = dram.tile([WSH], F32, tag="gin")
        gout = dram.tile([WTOT], F32, tag="gout")
        nc.sync.dma_start(gin[:], d["wflat_sh"][:])
        nc.gpsimd.collective_compute(
            "AllGather", mybir.AluOpType.bypass,
            replica_groups=[list(range(NCORES))],
            ins=[gin.opt()], outs=[gout.opt()])
        gathered = {}
        for name in W_ORDER:
            k, o = W_DIMS[name]
            off = W_OFF[name]
            gathered[name] = gout[off:off + k * o].rearrange(
                "(c p o) -> p c o", p=P, o=o)

        def wslab_b(name, k, n0, nw):
            """(b)-form moving slab [P, 1, nw] from wT rows [k*128, +128)."""
            t = wts.tile([P, 1, nw], F32, tag="wb")
            nc.sync.dma_start(t[:], gathered[name][:, k:k + 1, n0:n0 + nw])
            return t

        def wslab_a(name, k0, kc, m0, mw):
            """(a)-form stationary slab [P, kc<=4, mw] (k-chunks k0..k0+kc)."""
            t = wts.tile([P, kc, mw], F32, tag="wa")
            nc.sync.dma_start(t[:], gathered[name][:, k0:k0 + kc, m0:m0 + mw])
            return t

        # ---------------- phase A1: ci transpose, l1, cx1 chain --------------
        def phase_a1(g, st):
            col0 = g * BG
            ciT = shared.tile([P, KIN, BG], F32, tag="big16", name="ciT")
            st["ciT"] = ciT
            taskT = big.tile([5, BG], F32, tag="taskT")
            st["taskT"] = taskT
            tTASK = small.tile([P, GT, 5], F32, tag="tTASK")
            nc.sync.dma_start(tTASK[:, :, 0:4], task_r[:, g * GT:(g + 1) * GT, :])
            nc.gpsimd.memset(tTASK[:, :, 4:5], 1.0)
            yield
            # transpose ci into [feature-part, row] layout
            for t in range(GT):
                sROW = dbuf.tile([P, 1024], F32, tag="sROW")
                nc.sync.dma_start(sROW[:], state_r[:, g * GT + t, :])
                for c0 in (0, 4):
                    ps = pst.tile([P, 4 * P], F32, tag="pst")
                    for c in range(c0, c0 + 4):
                        nc.tensor.transpose(
                            ps[:, (c - c0) * P:(c - c0 + 1) * P],
                            sROW[:, c * P:(c + 1) * P], ident[:])
                    dst = ciT[:, c0:c0 + 4, t * P:(t + 1) * P]
                    src = ps[:].rearrange("p (c q) -> p c q", q=P)
                    nc.vector.tensor_copy(dst, src)
                    yield
                pt = pst.tile([P, 4 * P], F32, tag="pst")
                nc.tensor.transpose(pt[0:5, 0:P], tTASK[:, t, :], ident[:])
                nc.vector.tensor_copy(taskT[0:5, t * P:(t + 1) * P],
                                      pt[0:5, 0:P])
                yield

            # ---- l1 (b): z1 [P, GT, 1024]
            z1 = shared.tile([P, GT, HID], F32, tag="z1", name="z1")
            st["z1"] = z1
            for n0 in range(0, HID, 512):
                ps = psb.tile([P, GT, 512], F32, tag="psb")
                for k in range(KIN):
                    wb = wslab_b("l1", k, n0, 512)
                    for t in range(GT):
                        nc.tensor.matmul(
                            ps[:, t, :], ciT[:, k, t * P:(t + 1) * P],
                            wb[:, 0, :], start=(k == 0), stop=False)
                    yield
                for t in range(GT):
                    nc.tensor.matmul(
                        ps[:, t, :], taskT[0:5, t * P:(t + 1) * P],
                        tails["l1"][0:5, n0:n0 + 512], start=False, stop=True)
                yield
                for t in range(GT):
                    nc.vector.tensor_copy(z1[:, t, n0:n0 + 512], ps[:, t, :])
                yield

            # ---- cx1 chain -> kk0
            yield from cx_chain(g, st, 0)

        # ---------------- phase A2: cx2/cx3 chains ---------------------------
        def phase_a2(g, st):
            yield from cx_chain(g, st, 1)
            yield from cx_chain(g, st, 2)

        CX_DEFS = [("cx11", "cx12", HID, 8), ("cx21", "cx22", HID2, 4),
                   ("cx31", "cx32", HEADS, 1)]

        def cx_chain(g, st, cn):
            ciT = st["ciT"]
            taskT = st["taskT"]
            if True:
                pre, post, hidn, mch = CX_DEFS[cn]
                kc_pre = KIN
                httag = {0: "hx1", 1: "hx2", 2: "hx3"}[cn]
                hT = shared.tile([P, mch, BG], F32, tag=httag, name=f"hT{cn}")
                for m in range(mch):
                    ps = psa.tile([P, BG], F32, tag="psa")
                    for k0 in range(0, kc_pre, 4):
                        wa = wslab_a(pre, k0, 4, m * P, P)
                        for k in range(k0, k0 + 4):
                            nc.tensor.matmul(ps[:], wa[:, k - k0, :],
                                             ciT[:, k, :],
                                             start=(k == 0), stop=False)
                    nc.tensor.matmul(ps[:], tails[pre][0:5, m * P:(m + 1) * P],
                                     taskT[0:5, :], start=False, stop=True)
                    nc.scalar.activation(hT[:, m, :], ps[:], AF.Tanh,
                                         bias=zbias[:], scale=1.0)
                    yield

                # second layer (b) + incremental argmax
                kk = small.tile([P, GT], F32, tag=f"kk{cn}", name="kk")
                st[f"kk{cn}"] = kk
                kin2, out2 = HID_LAYERS[post]
                bestm = small.tile([P, GT], F32, tag="bestm")
                kkA = small.tile([P, GT], F32, tag="kkA")
                n0s = list(range(0, out2, 512))
                for ci_, n0 in enumerate(n0s):
                    nw = min(512, out2)
                    ps = psb.tile([P, GT, 512], F32, tag="psb")
                    for k in range(mch):
                        wb = wslab_b(post, k, n0, nw)
                        for t in range(GT):
                            nc.tensor.matmul(
                                ps[:, t, 0:nw], hT[:, k, t * P:(t + 1) * P],
                                wb[:, 0, :], start=(k == 0), stop=(k == mch - 1))
                        yield
                    m8 = small.tile([P, 8], F32, tag="am8")
                    idx = small.tile([P, 8], U32, tag="aidx")
                    idxf = small.tile([P, 8], F32, tag="aidxf")
                    for t in range(GT):
                        zcx = big.tile([P, 512], F32, tag="zcx", name="zcx")
                        nc.vector.scalar_tensor_tensor(
                            zcx[:, 0:nw], ps[:, t, 0:nw], 1.0,
                            breps[post][:, n0:n0 + nw], op0=OP.mult, op1=OP.add)
                        nc.vector.max(out=m8[:], in_=zcx[:, 0:nw])
                        nc.vector.max_index(idx[:], m8[:], zcx[:, 0:nw])
                        nc.vector.tensor_copy(idxf[:, 0:1], idx[:, 0:1])
                        if ci_ == 0 and len(n0s) == 1:
                            nc.vector.tensor_copy(kk[:, t:t + 1], idxf[:, 0:1])
                        elif ci_ == 0:
                            nc.vector.tensor_copy(kkA[:, t:t + 1], idxf[:, 0:1])
                            nc.vector.tensor_copy(bestm[:, t:t + 1],
                                                  m8[:, 0:1])
                        else:
                            gtu = small.tile([P, 1], U8, tag="agt")
                            nc.vector.tensor_tensor(
                                gtu[:], m8[:, 0:1], bestm[:, t:t + 1],
                                op=OP.is_gt)
                            i2 = small.tile([P, 1], F32, tag="ai2")
                            nc.vector.tensor_scalar(
                                i2[:], idxf[:, 0:1], float(n0), None,
                                op0=OP.add)
                            nc.vector.select(kk[:, t:t + 1], gtu[:], i2[:],
                                             kkA[:, t:t + 1])
                        yield

        # ---------------- kwta bisection ------------------------------------
        def kwta(zg, xg, kk, n):
            I = ITERS[n]
            loA = small.tile([P, GT], F32, tag="kwloA")
            loB = small.tile([P, GT], F32, tag="kwloB")
            hiA = small.tile([P, GT], F32, tag="kwhiA")
            hiB = small.tile([P, GT], F32, tag="kwhiB")
            chA = small.tile([P, GT], F32, tag="kwchA")
            chB = small.tile([P, GT], F32, tag="kwchB")
            cnt = small.tile([P, GT], F32, tag="kwcnt")
            kp1 = small.tile([P, GT], F32, tag="kwkp1")
            msum = small.tile([P, GT], F32, tag="kwmsum")
            mid = small.tile([P, GT], F32, tag="kwmid")
            nbias = small.tile([P, GT], F32, tag="kwnb")
            mn = small.tile([P, GT], F32, tag="kwmn")
            selu = small.tile([P, GT], U8, tag="kwselu")
            trash = scr.tile([P, n], BF16, tag=f"kwA{n}", name="trash")
            trashD = scr.tile([P, n], U8, tag=f"kwB{n}", name="trashD")

            nc.gpsimd.tensor_scalar(kp1[:], kk[:], 1.0, None, op0=OP.add)
            nc.gpsimd.memset(chA[:], 0.0)
            for t in range(GT):
                nc.vector.reduce_max(hiA[:, t:t + 1], zg[:, t, :], axis=AX.X)
                nc.vector.tensor_reduce(out=mn[:, t:t + 1], in_=zg[:, t, :],
                                        op=OP.min, axis=AX.X)
            nc.gpsimd.tensor_scalar(loA[:], mn[:], 1.0, None, op0=OP.subtract)
            yield

            lo, hi, ch = loA, hiA, chA
            lon, hin, chn = loB, hiB, chB
            for it in range(I):
                nc.gpsimd.tensor_tensor(msum[:], lo[:], hi[:], op=OP.add)
                nc.gpsimd.tensor_scalar(mid[:], msum[:], 0.5, None,
                                        op0=OP.mult)
                nc.gpsimd.tensor_scalar(nbias[:], mid[:], -SCALE, None,
                                        op0=OP.mult)
                for t in range(GT):
                    nc.scalar.activation(
                        trash[:], zg[:, t, :], AF.Sigmoid,
                        bias=nbias[:, t:t + 1], scale=SCALE,
                        accum_out=cnt[:, t:t + 1])
                nc.vector.tensor_tensor(selu[:], cnt[:], kp1[:], op=OP.is_ge)
                nc.vector.select(lon[:], selu[:], mid[:], lo[:])
                nc.vector.select(hin[:], selu[:], hi[:], mid[:])
                nc.vector.select(chn[:], selu[:], ch[:], cnt[:])
                lo, lon = lon, lo
                hi, hin = hin, hi
                ch, chn = chn, ch
                yield

            chii = small.tile([P, GT], I32, tag="kwchii")
            nc.vector.tensor_scalar(chn[:], ch[:], 0.25, None, op0=OP.subtract)
            nc.vector.tensor_copy(chii[:], chn[:])
            nc.vector.tensor_copy(ch[:], chii[:])
            rm1 = small.tile([P, GT], F32, tag="kwrm1")
            nc.vector.tensor_tensor(rm1[:], kk[:], ch[:], op=OP.subtract)
            yield

            for t in range(GT):
                m1 = scr.tile([P, n], F32, tag=f"kwA{n}", name="m1")
                gu8 = scr.tile([P, n], U8, tag=f"kwgu{n}", name="gu8")
                msk = scr.tile([P, n], F32, tag=f"kwmsk{n}", name="msk")
                nc.gpsimd.tensor_scalar(m1[:], zg[:, t, :], lo[:, t:t + 1],
                                        None, op0=OP.max)
                nc.vector.tensor_scalar(gu8[:], zg[:, t, :], hi[:, t:t + 1],
                                        None, op0=OP.is_gt)
                nc.vector.select(msk[:], gu8[:], negbig[:].to_broadcast([P, n]),
                                 m1[:])
                m8 = small.tile([P, 8], F32, tag="kwm8")
                nc.vector.max(out=m8[:], in_=msk[:])
                eq = small.tile([P, 8], F32, tag="kweq")
                nc.vector.tensor_scalar(eq[:], iota8[:], rm1[:, t:t + 1],
                                        None, op0=OP.is_equal)
                pr = small.tile([P, 8], F32, tag="kwpr")
                nc.vector.tensor_tensor(pr[:], eq[:], m8[:], op=OP.mult)
                u = small.tile([P, 1], F32, tag="kwu")
                nc.vector.reduce_sum(u[:], pr[:], axis=AX.X)
                yield
                geu = scr.tile([P, n], U8, tag=f"kwgu{n}", name="geu")
                nc.vector.tensor_scalar(geu[:], zg[:, t, :], u[:], None,
                                        op0=OP.is_gt)
                zth = scr.tile([P, n], F32, tag=f"kwA{n}", name="zth")
                nc.gpsimd.tensor_scalar(zth[:], zg[:, t, :], THIRD, None,
                                        op0=OP.mult)
                nc.vector.select(xg[:, t, :], geu[:], zg[:, t, :], zth[:])
                yield

        # transpose [P, GT, n] -> xT [P, n//P, BG]
        def transpose_x(xg, xT, n):
            nch = n // P
            for t in range(GT):
                for c0 in range(0, nch, 4):
                    cw = min(4, nch - c0)
                    ps = pst.tile([P, 4 * P], F32, tag="pst")
                    for c in range(c0, c0 + cw):
                        nc.tensor.transpose(
                            ps[:, (c - c0) * P:(c - c0 + 1) * P],
                            xg[:, t, c * P:(c + 1) * P], ident[:])
                    dst = xT[:, c0:c0 + cw, t * P:(t + 1) * P]
                    src = ps[:, 0:cw * P].rearrange("p (c q) -> p c q", q=P)
                    nc.vector.tensor_copy(dst, src)
                    yield

        # ---------------- phase B1: kwta1, x1T, l2 ---------------------------
        def phase_b1(g, st):
            x1 = shared.tile([P, GT, HID], F32, tag="big16", name="x1")
            yield from kwta(st["z1"], x1, st["kk0"], HID)
            x1T = shared.tile([P, HID // P, BG], F32, tag="hx1", name="x1T")
            yield from transpose_x(x1, x1T, HID)
            z2 = shared.tile([P, GT, HID2], F32, tag="z2")
            st["z2"] = z2
            ps = psb.tile([P, GT, 512], F32, tag="psb")
            for k in range(HID // P):
                wb = wslab_b("l2", k, 0, HID2)
                for t in range(GT):
                    nc.tensor.matmul(
                        ps[:, t, :], x1T[:, k, t * P:(t + 1) * P],
                        wb[:, 0, :], start=(k == 0), stop=(k == HID // P - 1))
                yield
            for t in range(GT):
                nc.vector.scalar_tensor_tensor(
                    z2[:, t, :], ps[:, t, :], 1.0, breps["l2"][:],
                    op0=OP.mult, op1=OP.add)
            yield

        # ---------------- phase B2: kwta2, x2T, l3 ---------------------------
        def phase_b2(g, st):
            x2 = big.tile([P, GT, HID2], F32, tag="x2")
            yield from kwta(st["z2"], x2, st["kk1"], HID2)
            x2T = shared.tile([P, HID2 // P, BG], F32, tag="hx2", name="x2T")
            yield from transpose_x(x2, x2T, HID2)
            ps3 = psa.tile([P, BG], F32, tag="psa")
            wa = wslab_a("l3", 0, HID2 // P, 0, P)
            for k in range(HID2 // P):
                nc.tensor.matmul(ps3[:], wa[:, k, :], x2T[:, k, :],
                                 start=(k == 0), stop=(k == HID2 // P - 1))
            z3T = big.tile([P, BG], F32, tag="zot", name="z3T")
            nc.vector.scalar_tensor_tensor(
                z3T[:], ps3[:], 1.0, bcols["l3"][:].to_broadcast([P, BG]),
                op0=OP.mult, op1=OP.add)
            yield
            z3 = shared.tile([P, GT, HEADS], F32, tag="z3")
            st["z3"] = z3
            for t in range(GT):
                pt = pst.tile([P, 4 * P], F32, tag="pst")
                nc.tensor.transpose(pt[:, 0:P], z3T[:, t * P:(t + 1) * P],
                                    ident[:])
                nc.vector.tensor_copy(z3[:, t, :], pt[:, 0:P])
            yield

        # ---------------- phase B3: kwta3, x3T, l4, out ----------------------
        def phase_b3(g, st):
            col0 = g * BG
            x3 = big.tile([P, GT, HEADS], F32, tag="x3")
            yield from kwta(st["z3"], x3, st["kk2"], HEADS)
            x3T = shared.tile([P, 1, BG], F32, tag="hx3", name="x3T")
            yield from transpose_x(x3, x3T, HEADS)
            ps4 = psa.tile([P, BG], F32, tag="psa")
            wa = wslab_a("l4", 0, 1, 0, P)
            nc.tensor.matmul(ps4[:], wa[:, 0, :], x3T[:, 0, :],
                             start=True, stop=True)
            og = big.tile([P, BG], F32, tag="zot", name="og")
            nc.vector.scalar_tensor_tensor(
                og[:], ps4[:], 1.0, bcols["l4"][:].to_broadcast([P, BG]),
                op0=OP.mult, op1=OP.add)
            nc.sync.dma_start(outT[:, col0:col0 + BG], og[:])
            yield

        # ---------------- weave ------------------------------------------
        sts = [dict() for _ in range(NG)]

        def weave(gens):
            active = list(gens)
            while active:
                keep = []
                for it in active:
                    try:
                        next(it)
                        keep.append(it)
                    except StopIteration:
                        pass
                active = keep

        def phase_a(g, st):
            yield from phase_a1(g, st)
            yield from phase_a2(g, st)

        def seq(*gens):
            for gi in gens:
                yield from gi

        slots = [
            [phase_a(0, sts[0])],
            [phase_a(1, sts[1])],
            [phase_a(2, sts[2]), phase_b1(0, sts[0])],
            [phase_a(3, sts[3]), phase_b2(0, sts[0]), phase_b1(1, sts[1])],
            [phase_b3(0, sts[0]), phase_b2(1, sts[1]),
             seq(phase_b1(2, sts[2]), phase_b1(3, sts[3]))],
            [phase_b3(1, sts[1]),
             seq(phase_b2(2, sts[2]), phase_b2(3, sts[3]))],
            [seq(phase_b3(2, sts[2]), phase_b3(3, sts[3]))],
        ]
        for s in slots:
            weave(s)


# ----------------------------------------------------------------------------
# host wrapper
# ----------------------------------------------------------------------------

_CACHE = {}


def _get_program():
    if "nc" not in _CACHE:
        _CACHE["nc"] = build_program()
    return _CACHE["nc"]


def _fingerprint(arrs):
    out = []
    for a in arrs:
        out.append((id(a), a.shape, a.dtype.str,
                    float(a.flat[0]), float(a.flat[-1])))
    return tuple(out)


def _prep_weights(ws):
    """ws: dict name -> (w, b). Returns the replicated input map (cached)."""
    arrs = [a for pair in ws.values() for a in pair]
    key = _fingerprint(arrs)
    hit = _CACHE.get("wkey")
    if hit == key:
        return _CACHE["wmap"]
    m = {}
    shards = {}
    for name, (w, b) in ws.items():
        w = np.asarray(w, dtype=np.float32)
        b = np.asarray(b, dtype=np.float32)
        if name in IN_LAYERS:
            wT = np.ascontiguousarray(w[:, :1024].T)
            m[f"{name}_tail"] = np.ascontiguousarray(
                np.vstack([w[:, 1024:1028].T, b[None, :]]))
        else:
            wT = np.ascontiguousarray(w.T)
            if name in ("l3", "l4"):
                m[f"{name}_bcol"] = np.ascontiguousarray(
                    np.broadcast_to(b[:, None], (P, 1)))
            else:
                m[f"{name}_brep"] = np.ascontiguousarray(
                    np.broadcast_to(b[None, :], (P, b.shape[0])))
        shards[name] = wT
    wflat = np.concatenate([shards[n].reshape(-1) for n in W_ORDER])
    wsh = [wflat[c * WSH:(c + 1) * WSH] for c in range(NCORES)]
    _CACHE["wkey"] = key
    _CACHE["wmap"] = (m, {"wflat_sh": wsh})
    return m, {"wflat_sh": wsh}


def kernel(**inputs):
    _trace = bool(inputs.pop("_trace", False))
    nc = _get_program()
    state = np.asarray(inputs["state"], dtype=np.float32)
    task = np.asarray(inputs["task_indicator"], dtype=np.float32)
    ws = {n: (inputs[f"{n}_w"], inputs[f"{n}_b"])
          for n in list(IN_LAYERS) + list(HID_LAYERS)}
    common, shards = _prep_weights(ws)
    in_maps = []
    for c in range(NCORES):
        m = dict(common)
        m["state"] = state[c * BC:(c + 1) * BC]
        m["task"] = task[c * BC:(c + 1) * BC]
        for sk, sv in shards.items():
            m[sk] = sv[c]
        in_maps.append(m)
    res = run_bass_kernel_spmd(nc, in_maps, core_ids=list(range(NCORES)),
                               trace=_trace)
    kernel.last_exec_time_ns = res.exec_time_ns
    out = np.concatenate([r["outT"].T for r in res.results], axis=0)
    return np.ascontiguousarray(out, dtype=np.float32)


kernel.last_exec_time_ns = None
